# revision 20
# baseline (speedup 1.0000x reference)
"""Trainium2 Bass kernel for LogWeightedDICELossMultiClass3D.

Input: output (4,3,64,192,192) f32, masks (same), loss_threshold scalar.

Strategy: host packs both inputs into ONE bf16 tensor
    c = copysign(output, 0.5 - masks)        (sign bit = mask, |c| = output)
so DMA traffic is 2 bytes/voxel instead of 8. H=192 is sharded into 8
slabs of 24 rows (one per core, 1-row halo clamped on host).

Per core, 6 supertiles of 128 partitions (= 2 volumes x 64 z), free dim =
26 H-rows x 192 W. Reductions are expensive on this HW (any DVE op with
accum_out drops to 1x; ACT is 1 elem/cycle; PE ones-matmul is 1 col/cycle)
so each of the six per-(vol,z)/per-volume sums rides the cheapest slot:
  sum_o    : fused accum of the ACT Abs pass that materializes a=|c|
  sum_c    : PE ones-matmul on c  (sum_om = (sum_o - sum_c)/2 on host)
  sum_ts   : PE ones-matmul on ts (both accumulate over all supertiles
             into one [12,512] PSUM tile via a block ones lhsT)
  sum_m    : DVE is_lt + add-accum (1x)
  sum_tsm  : DVE is_lt + add-accum for 3 supertiles, ACT saturated
             sigmoid for 3 (engine balance)
  sum_edge : ts = (a > thr) on DVE (4x); d = W-deriv(ts) on DVE (2x);
             H-smooth (3 shifted taps) x Z-smooth (block-banded lhsT) on
             PE into PSUM; edge = (grad > 0) via saturated Sigmoid on
             ACT + accum (grad is integer-valued so this is exact)
Host combines the tiny partials into the scalar loss
(sum_eq = vox - sum_ts - sum_m + 2*sum_tsm).
"""

import numpy as np
import ml_dtypes

import concourse.bacc as bacc
import concourse.bass as bass
import concourse.tile as tile
from concourse import mybir
from concourse.bass_utils import run_bass_kernel_spmd

F32 = mybir.dt.float32
BF16 = mybir.dt.bfloat16
ALU = mybir.AluOpType
ACTF = mybir.ActivationFunctionType

B, C, Z, H, W = 4, 3, 64, 192, 192
NV = B * C            # 12 volumes
NCORES = 8
HC = H // NCORES      # 24 H-rows per core
NS = NV // 2          # 6 supertiles (2 volumes each)
FH = HC + 2           # 26 rows incl halo
FW = FH * W           # 4992 free elements per partition of c / a / ts / d
CW = HC * W           # 4608 center free elements
C0 = W                # flat offset of center region (row 1)
GRP = 768             # tap-group width (2 PSUM banks)
VOX = Z * H * W
ACT_TSM = ()          # supertiles whose tsm-count runs on ACT, not DVE

_CACHE = {}


def _band64():
    """[1,2,1] Z-smoothing matrix with scipy 'reflect' ends."""
    M = np.zeros((Z, Z), dtype=np.float64)
    for i in range(Z):
        M[i, i] = 2.0
        M[i, max(i - 1, 0)] += 1.0
        M[i, min(i + 1, Z - 1)] += 1.0
    return M


def _consts():
    Bz = _band64()
    blk = np.zeros((128, 128), dtype=np.float64)
    blk[:64, :64] = Bz
    blk[64:, 64:] = Bz
    bz1 = blk.astype(ml_dtypes.bfloat16)          # weights 1,2,3 - exact
    bz2 = (2.0 * blk).astype(ml_dtypes.bfloat16)  # weights 2,4,6 - exact
    # per-supertile [128,12] ones blocks: slice s has col 2s = ones on
    # partitions 0..63 (volume 2s) and col 2s+1 = ones on 64..127.
    ones12 = np.zeros((128, 12 * NS), dtype=ml_dtypes.bfloat16)
    for s in range(NS):
        ones12[:64, 12 * s + 2 * s] = 1.0
        ones12[64:, 12 * s + 2 * s + 1] = 1.0
    return bz1, bz2, ones12


def _build_program():
    nc = bacc.Bacc("TRN2", target_bir_lowering=False, debug=False,
                   num_devices=NCORES)
    c_d = nc.dram_tensor("c", [NV * Z, FW], BF16, kind="ExternalInput").ap()
    thr_d = nc.dram_tensor("thr", [1, 1], F32, kind="ExternalInput").ap()
    bz1_d = nc.dram_tensor("bz1", [128, 128], BF16, kind="ExternalInput").ap()
    bz2_d = nc.dram_tensor("bz2", [128, 128], BF16, kind="ExternalInput").ap()
    ones_d = nc.dram_tensor("ones12", [128, 12 * NS], BF16,
                            kind="ExternalInput").ap()
    part_d = nc.dram_tensor("partials", [128, 48], F32,
                            kind="ExternalOutput").ap()
    osum_d = nc.dram_tensor("osum", [12, 2048], F32,
                            kind="ExternalOutput").ap()

    from contextlib import ExitStack
    with tile.TileContext(nc) as tc, ExitStack() as ctx:
        consts = ctx.enter_context(tc.tile_pool(name="consts", bufs=1))
        io = ctx.enter_context(tc.tile_pool(name="io", bufs=4))
        mida = ctx.enter_context(tc.tile_pool(name="mida", bufs=2))
        midd = ctx.enter_context(tc.tile_pool(name="midd", bufs=3))
        scr = ctx.enter_context(tc.tile_pool(name="scr", bufs=3))
        slots = ctx.enter_context(tc.tile_pool(name="slots", bufs=1))
        gps = ctx.enter_context(tc.tile_pool(name="gps", bufs=2, space="PSUM"))
        ops = ctx.enter_context(tc.tile_pool(name="ops", bufs=1, space="PSUM"))

        thr_t = consts.tile([128, 1], F32)
        nc.gpsimd.dma_start(out=thr_t, in_=thr_d.to_broadcast([128, 1]))
        bz1_t = consts.tile([128, 128], BF16)
        nc.default_dma_engine.dma_start(out=bz1_t, in_=bz1_d)
        bz2_t = consts.tile([128, 128], BF16)
        nc.default_dma_engine.dma_start(out=bz2_t, in_=bz2_d)
        ones_t = consts.tile([128, 12 * NS], BF16)
        nc.default_dma_engine.dma_start(out=ones_t, in_=ones_d)
        nbias_t = consts.tile([128, 1], F32)
        nc.vector.memset(nbias_t, -50.0)
        negthr_t = consts.tile([128, 1], F32)
        nc.vector.tensor_scalar(out=negthr_t, in0=thr_t, scalar1=-1.0,
                                scalar2=None, op0=ALU.mult)
        # bias for ACT-side tsm count: sigmoid(-1e8*c - 1e8*thr)
        bsig_t = consts.tile([128, 1], F32)
        nc.vector.tensor_scalar(out=bsig_t, in0=thr_t, scalar1=-1.0e8,
                                scalar2=None, op0=ALU.mult)

        tsmsum = slots.tile([128, NS], F32)
        osum_t = slots.tile([128, NS], F32)
        zero128_t = slots.tile([128, 128], BF16)
        nc.vector.memset(zero128_t, 0.0)
        nc.vector.memset(tsmsum, 0.0)
        edgesum = slots.tile([128, 6 * NS], F32)
        osb = slots.tile([12, 2048], F32)

        # one PSUM bank each, accumulated across all supertiles
        opc_t = ops.tile([12, 512], F32, name="opc")
        opt_t = ops.tile([12, 512], F32, name="opt")
        opm_t = ops.tile([12, 512], F32, name="opm")
        opq_t = ops.tile([12, 512], F32, name="opq")

        state = {}
        ctiles = []

        def front(s):
            c_t = io.tile([128, FW], BF16, tag="c", name=f"c{s}")
            nc.default_dma_engine.dma_start(
                out=c_t, in_=c_d[128 * s:128 * (s + 1), :])
            cc = c_t[:, C0:C0 + CW]
            c3 = c_t.rearrange("p (a b) -> p a b", b=W)
            o12 = ones_t[:, 12 * s:12 * (s + 1)]

            # sum(c) over center: ones-matmul accumulating across supertiles
            for k in range(9):
                nc.tensor.matmul(
                    out=opc_t, lhsT=o12,
                    rhs=c_t[:, C0 + 512 * k:C0 + 512 * (k + 1)],
                    start=(s == 0 and k == 0), stop=(s == NS - 1 and k == 8),
                    skip_group_check=True)

            # a = |c|: ACT Abs on center with fused sum(o); halo rows on DVE
            a_t = mida.tile([128, FW], BF16, tag="a", name=f"a{s}")
            a3 = a_t.rearrange("p (a b) -> p a b", b=W)
            nc.scalar.activation(
                out=a_t[:, C0:C0 + CW], in_=cc, func=ACTF.Abs,
                accum_out=osum_t[:, s:s + 1])
            nc.vector.scalar_tensor_tensor(
                out=a3[:, 0:1, :], in0=c3[:, 0:1, :], scalar=-1.0,
                in1=c3[:, 0:1, :], op0=ALU.mult, op1=ALU.max)
            nc.vector.scalar_tensor_tensor(
                out=a3[:, FH - 1:FH, :], in0=c3[:, FH - 1:FH, :], scalar=-1.0,
                in1=c3[:, FH - 1:FH, :], op0=ALU.mult, op1=ALU.max)

            # ts = (a > thr), whole tile in one 4x op (no accum!)
            ts_t = mida.tile([128, FW], BF16, tag="ts", name=f"ts{s}")
            ts3 = ts_t.rearrange("p (a b) -> p a b", b=W)
            nc.vector.tensor_scalar(
                out=ts_t, in0=a_t, scalar1=thr_t, scalar2=None,
                op0=ALU.is_gt)

            # sum(ts) over center: ones-matmul accumulating across supertiles
            for k in range(9):
                nc.tensor.matmul(
                    out=opt_t, lhsT=o12,
                    rhs=ts_t[:, C0 + 512 * k:C0 + 512 * (k + 1)],
                    start=(s == 0 and k == 0), stop=(s == NS - 1 and k == 8),
                    skip_group_check=True)

            # d = W-derivative of ts (symmetric boundary), cols permuted:
            # cols 0..189 = d[w=1..190], col 190 = d[w=0], col 191 = d[w=191]
            d_t = midd.tile([128, FW], BF16, tag="d", name=f"d{s}")
            d3 = d_t.rearrange("p (a b) -> p a b", b=W)
            nc.vector.tensor_tensor(
                out=d3[:, :, 0:190], in0=ts3[:, :, 2:192],
                in1=ts3[:, :, 0:190], op=ALU.subtract)
            nc.vector.tensor_tensor(
                out=d3[:, :, 190:191], in0=ts3[:, :, 1:2],
                in1=ts3[:, :, 0:1], op=ALU.subtract)
            nc.vector.tensor_tensor(
                out=d3[:, :, 191:192], in0=ts3[:, :, 191:192],
                in1=ts3[:, :, 190:191], op=ALU.subtract)
            state[s] = (c_t, d_t)
            ctiles.append(c_t)

        def back(s):
            c_t, d_t = state.pop(s)
            cc = c_t[:, C0:C0 + CW]
            o12 = ones_t[:, 12 * s:12 * (s + 1)]

            # counts: materialize indicator at 4x, two exact bf16 pair-folds
            # at 2x, then a 3-matmul ones-reduce on PE (all correctly priced
            # by the scheduler's cost model, unlike fused DVE accum at 1x)
            def count_reduce(tag, op_psum, scalar1):
                ind = scr.tile([128, CW], BF16, tag="dum", name=f"{tag}i{s}")
                nc.vector.tensor_scalar(
                    out=ind, in0=cc, scalar1=scalar1, scalar2=None,
                    op0=ALU.is_lt)
                f1 = scr.tile([128, CW // 2], BF16, tag="f1", name=f"{tag}f{s}")
                nc.vector.tensor_tensor(
                    out=f1, in0=ind[:, 0:CW // 2], in1=ind[:, CW // 2:CW],
                    op=ALU.add)
                f2 = scr.tile([128, CW // 4], BF16, tag="f2", name=f"{tag}g{s}")
                nc.vector.tensor_tensor(
                    out=f2, in0=f1[:, 0:CW // 4], in1=f1[:, CW // 4:CW // 2],
                    op=ALU.add)
                for kk, (o0, w) in enumerate([(0, 512), (512, 512),
                                              (1024, 128)]):
                    nc.tensor.matmul(
                        out=op_psum[:, 0:w], lhsT=o12,
                        rhs=f2[:, o0:o0 + w],
                        start=(s == 0 and kk == 0),
                        stop=(s == NS - 1 and kk == 2),
                        skip_group_check=True)

            count_reduce("m", opm_t, 0.0)
            if s in ACT_TSM:
                dtsm = scr.tile([128, CW], BF16, tag="dum", name=f"dtsm{s}")
                nc.scalar.activation(
                    out=dtsm, in_=cc, func=ACTF.Sigmoid,
                    scale=-1.0e8, bias=bsig_t,
                    accum_out=tsmsum[:, s:s + 1])
                if s == NS - 1:
                    # close the opq accumulation group on the last supertile
                    nc.tensor.matmul(
                        out=opq_t[:, 0:128], lhsT=o12,
                        rhs=zero128_t, start=False, stop=True,
                        skip_group_check=True)
            else:
                count_reduce("q", opq_t, negthr_t)

            # grad = S_z(S_h(d)) via 3 H-shifted banded matmuls into PSUM
            for j in range(6):
                g_t = gps.tile([128, GRP], F32, tag="g", name=f"g{s}_{j}")
                base = C0 + GRP * j
                for di, (lhs, doff) in enumerate(
                        [(bz1_t, -W), (bz1_t, W), (bz2_t, 0)]):
                    for k, (o0, w) in enumerate([(0, 512), (512, 256)]):
                        off = base + o0 + doff
                        nc.tensor.matmul(
                            out=g_t[:, o0:o0 + w],
                            lhsT=lhs, rhs=d_t[:, off:off + w],
                            start=(di == 0), stop=(di == 2))
                # edge = (grad > 0): integer grad, sigmoid saturates
                e_t = scr.tile([128, GRP], BF16, tag="edge", name=f"e{s}_{j}")
                nc.scalar.activation(
                    out=e_t, in_=g_t, func=ACTF.Sigmoid,
                    scale=100.0, bias=nbias_t,
                    accum_out=edgesum[:, 6 * s + j:6 * s + j + 1])

        for s in range(NS):
            front(s)
            if s >= 1:
                back(s - 1)
        back(NS - 1)

        nc.vector.tensor_copy(out=osb[:, 0:512], in_=opc_t)
        nc.vector.tensor_copy(out=osb[:, 512:1024], in_=opt_t)
        nc.vector.tensor_copy(out=osb[:, 1024:1536], in_=opm_t)
        nc.vector.tensor_copy(out=osb[:, 1536:2048], in_=opq_t)

        nc.default_dma_engine.dma_start(out=part_d[:, 0:6], in_=osum_t)
        nc.default_dma_engine.dma_start(out=part_d[:, 6:12], in_=tsmsum)
        nc.default_dma_engine.dma_start(out=part_d[:, 12:48], in_=edgesum)
        nc.default_dma_engine.dma_start(out=osum_d, in_=osb)

    nc.compile()
    return nc


def _get_program():
    if "nc" not in _CACHE:
        _CACHE["nc"] = _build_program()
    return _CACHE["nc"]


def _make_in_maps(output, masks, loss_threshold):
    o = np.asarray(output, dtype=np.float32)
    m = np.asarray(masks, dtype=np.float32)
    c = np.copysign(o, np.float32(0.5) - m).astype(ml_dtypes.bfloat16)
    c5 = c.reshape(NV, Z, H, W)
    thr = np.full((1, 1), np.float32(np.asarray(loss_threshold)), np.float32)
    bz1, bz2, ones12 = _consts()
    in_maps = []
    for cid in range(NCORES):
        h0 = HC * cid
        idx = np.clip(np.arange(h0 - 1, h0 + HC + 1), 0, H - 1)
        c_sh = np.ascontiguousarray(c5[:, :, idx, :]).reshape(NV * Z, FW)
        in_maps.append({
            "c": c_sh, "thr": thr,
            "bz1": bz1, "bz2": bz2, "ones12": ones12,
        })
    return in_maps


def _combine(results):
    """Host-side tiny reduction: per-core partials -> loss scalar."""
    sum_ts = np.zeros(NV)
    sum_tsm = np.zeros(NV)
    sum_m = np.zeros(NV)
    sum_o = np.zeros(NV)
    sum_edge = np.zeros(NV)
    sum_c = np.zeros(NV)
    for r in results:
        p = np.asarray(r["partials"], dtype=np.float64)
        osum = np.asarray(r["osum"], dtype=np.float64)
        # [partition, s]: volume = 2s + partition//64, z = partition%64
        sum_o += p[:, 0:6].reshape(2, 64, NS).sum(1).T.reshape(-1)
        sum_tsm += p[:, 6:12].reshape(2, 64, NS).sum(1).T.reshape(-1)
        sum_edge += (p[:, 12:48].reshape(2, 64, NS, 6).sum(axis=(1, 3))
                     .T.reshape(-1))
        # osum rows are volumes directly (block ones lhsT)
        sum_c += osum[:, 0:512].sum(-1)
        sum_ts += osum[:, 512:1024].sum(-1)
        sum_m += osum[:, 1024:1536].sum(-1)
        sum_tsm += osum[:, 1536:2048].sum(-1)

    sum_om = 0.5 * (sum_o - sum_c)
    sum_eq = VOX - sum_ts - sum_m + 2.0 * sum_tsm

    freq = (sum_m / VOX).reshape(B, C)
    med = np.median(freq, axis=1, keepdims=True)
    w0 = 2.0 * med / (freq.min(axis=1, keepdims=True) + 1e-5)
    cw = (med / (freq + 1e-5)) * sum_eq.reshape(B, C) \
        + w0 * sum_edge.reshape(B, C)
    ps1 = sum_om.reshape(B, C)
    ps2 = (sum_o + sum_m).reshape(B, C)
    nom = (cw * ps1).sum(1)
    denom = (cw * ps2 + 1e-7).sum(1)
    loss = (1.0 - 2.0 * nom / denom).sum() / B
    return np.array([loss], dtype=np.float32)


def run(output, masks, loss_threshold, trace=False, **trace_kwargs):
    nc = _get_program()
    in_maps = _make_in_maps(output, masks, loss_threshold)
    res = run_bass_kernel_spmd(nc, in_maps, list(range(NCORES)),
                               trace=trace, **trace_kwargs)
    return _combine(res.results), res


def kernel(output, masks, loss_threshold):
    loss, _ = run(output, masks, loss_threshold)
    return loss


# revision 22
# speedup vs baseline: 1.0552x; 1.0552x over previous
"""Trainium2 Bass kernel for LogWeightedDICELossMultiClass3D.

Input: output (4,3,64,192,192) f32, masks (same), loss_threshold scalar.

Strategy: host packs both inputs into ONE bf16 tensor
    c = copysign(output, 0.5 - masks)        (sign bit = mask, |c| = output)
so DMA traffic is 2 bytes/voxel instead of 8. H=192 is sharded into 8
slabs of 24 rows (one per core, 1-row halo clamped on host).

Per core, 6 supertiles of 128 partitions (= 2 volumes x 64 z), free dim =
26 H-rows x 192 W. Reductions are expensive on this HW (any DVE op with
accum_out drops to 1x; ACT is 1 elem/cycle; PE ones-matmul is 1 col/cycle)
so each of the six per-(vol,z)/per-volume sums rides the cheapest slot:
  sum_o    : fused accum of the ACT Abs pass that materializes a=|c|
  sum_c    : PE ones-matmul on c  (sum_om = (sum_o - sum_c)/2 on host)
  sum_ts   : PE ones-matmul on ts (both accumulate over all supertiles
             into one [12,512] PSUM tile via a block ones lhsT)
  sum_m    : DVE is_lt + add-accum (1x)
  sum_tsm  : DVE is_lt + add-accum for 3 supertiles, ACT saturated
             sigmoid for 3 (engine balance)
  sum_edge : ts = (a > thr) on DVE (4x); d = W-deriv(ts) on DVE (2x);
             H-smooth (3 shifted taps) x Z-smooth (block-banded lhsT) on
             PE into PSUM; edge = (grad > 0) via saturated Sigmoid on
             ACT + accum (grad is integer-valued so this is exact)
Host combines the tiny partials into the scalar loss
(sum_eq = vox - sum_ts - sum_m + 2*sum_tsm).
"""

import numpy as np
import ml_dtypes

import concourse.bacc as bacc
import concourse.bass as bass
import concourse.tile as tile
from concourse import mybir
from concourse.bass_utils import run_bass_kernel_spmd

F32 = mybir.dt.float32
BF16 = mybir.dt.bfloat16
ALU = mybir.AluOpType
ACTF = mybir.ActivationFunctionType

B, C, Z, H, W = 4, 3, 64, 192, 192
NV = B * C            # 12 volumes
NCORES = 8
HC = H // NCORES      # 24 H-rows per core
NS = NV // 2          # 6 supertiles (2 volumes each)
FH = HC + 2           # 26 rows incl halo
FW = FH * W           # 4992 free elements per partition of c / a / ts / d
CW = HC * W           # 4608 center free elements
C0 = W                # flat offset of center region (row 1)
GRP = 768             # tap-group width (2 PSUM banks)
VOX = Z * H * W
ACT_TSM = (4, 5)      # supertiles whose tsm-count runs on ACT, not DVE

_CACHE = {}


def _band64():
    """[1,2,1] Z-smoothing matrix with scipy 'reflect' ends."""
    M = np.zeros((Z, Z), dtype=np.float64)
    for i in range(Z):
        M[i, i] = 2.0
        M[i, max(i - 1, 0)] += 1.0
        M[i, min(i + 1, Z - 1)] += 1.0
    return M


def _consts():
    Bz = _band64()
    blk = np.zeros((128, 128), dtype=np.float64)
    blk[:64, :64] = Bz
    blk[64:, 64:] = Bz
    bz1 = blk.astype(ml_dtypes.bfloat16)          # weights 1,2,3 - exact
    bz2 = (2.0 * blk).astype(ml_dtypes.bfloat16)  # weights 2,4,6 - exact
    # per-supertile [128,12] ones blocks: slice s has col 2s = ones on
    # partitions 0..63 (volume 2s) and col 2s+1 = ones on 64..127.
    ones12 = np.zeros((128, 12 * NS), dtype=ml_dtypes.bfloat16)
    for s in range(NS):
        ones12[:64, 12 * s + 2 * s] = 1.0
        ones12[64:, 12 * s + 2 * s + 1] = 1.0
    return bz1, bz2, ones12


def _build_program():
    nc = bacc.Bacc("TRN2", target_bir_lowering=False, debug=False,
                   num_devices=NCORES)
    c_d = nc.dram_tensor("c", [NV * Z, FW], BF16, kind="ExternalInput").ap()
    thr_d = nc.dram_tensor("thr", [1, 1], F32, kind="ExternalInput").ap()
    bz1_d = nc.dram_tensor("bz1", [128, 128], BF16, kind="ExternalInput").ap()
    bz2_d = nc.dram_tensor("bz2", [128, 128], BF16, kind="ExternalInput").ap()
    ones_d = nc.dram_tensor("ones12", [128, 12 * NS], BF16,
                            kind="ExternalInput").ap()
    part_d = nc.dram_tensor("partials", [128, 48], F32,
                            kind="ExternalOutput").ap()
    osum_d = nc.dram_tensor("osum", [12, 2048], F32,
                            kind="ExternalOutput").ap()

    from contextlib import ExitStack
    with tile.TileContext(nc) as tc, ExitStack() as ctx:
        consts = ctx.enter_context(tc.tile_pool(name="consts", bufs=1))
        io = ctx.enter_context(tc.tile_pool(name="io", bufs=4))
        mida = ctx.enter_context(tc.tile_pool(name="mida", bufs=2))
        midd = ctx.enter_context(tc.tile_pool(name="midd", bufs=3))
        scr = ctx.enter_context(tc.tile_pool(name="scr", bufs=3))
        slots = ctx.enter_context(tc.tile_pool(name="slots", bufs=1))
        gps = ctx.enter_context(tc.tile_pool(name="gps", bufs=2, space="PSUM"))
        ops = ctx.enter_context(tc.tile_pool(name="ops", bufs=1, space="PSUM"))

        thr_t = consts.tile([128, 1], F32)
        nc.gpsimd.dma_start(out=thr_t, in_=thr_d.to_broadcast([128, 1]))
        bz1_t = consts.tile([128, 128], BF16)
        nc.default_dma_engine.dma_start(out=bz1_t, in_=bz1_d)
        bz2_t = consts.tile([128, 128], BF16)
        nc.default_dma_engine.dma_start(out=bz2_t, in_=bz2_d)
        ones_t = consts.tile([128, 12 * NS], BF16)
        nc.default_dma_engine.dma_start(out=ones_t, in_=ones_d)
        nbias_t = consts.tile([128, 1], F32)
        nc.vector.memset(nbias_t, -50.0)
        negthr_t = consts.tile([128, 1], F32)
        nc.vector.tensor_scalar(out=negthr_t, in0=thr_t, scalar1=-1.0,
                                scalar2=None, op0=ALU.mult)
        # bias for ACT-side tsm count: sigmoid(-1e8*c - 1e8*thr)
        bsig_t = consts.tile([128, 1], F32)
        nc.vector.tensor_scalar(out=bsig_t, in0=thr_t, scalar1=-1.0e8,
                                scalar2=None, op0=ALU.mult)

        tsmsum = slots.tile([128, NS], F32)
        osum_t = slots.tile([128, NS], F32)
        zero128_t = slots.tile([128, 128], BF16)
        nc.vector.memset(zero128_t, 0.0)
        nc.vector.memset(tsmsum, 0.0)
        edgesum = slots.tile([128, 6 * NS], F32)
        osb = slots.tile([12, 2048], F32)

        # one PSUM bank each, accumulated across all supertiles
        opc_t = ops.tile([12, 512], F32, name="opc")
        opt_t = ops.tile([12, 512], F32, name="opt")
        opm_t = ops.tile([12, 512], F32, name="opm")
        opq_t = ops.tile([12, 512], F32, name="opq")

        state = {}
        ctiles = []

        dma_engines = [nc.default_dma_engine, nc.gpsimd]

        def front(s):
            c_t = io.tile([128, FW], BF16, tag="c", name=f"c{s}")
            dma_engines[s % len(dma_engines)].dma_start(
                out=c_t, in_=c_d[128 * s:128 * (s + 1), :])
            cc = c_t[:, C0:C0 + CW]
            c3 = c_t.rearrange("p (a b) -> p a b", b=W)
            o12 = ones_t[:, 12 * s:12 * (s + 1)]

            # sum(c) over center: ones-matmul accumulating across supertiles
            for k in range(9):
                nc.tensor.matmul(
                    out=opc_t, lhsT=o12,
                    rhs=c_t[:, C0 + 512 * k:C0 + 512 * (k + 1)],
                    start=(s == 0 and k == 0), stop=(s == NS - 1 and k == 8),
                    skip_group_check=True)

            # a = |c|: ACT Abs on center with fused sum(o); halo rows on DVE
            a_t = mida.tile([128, FW], BF16, tag="a", name=f"a{s}")
            a3 = a_t.rearrange("p (a b) -> p a b", b=W)
            nc.scalar.activation(
                out=a_t[:, C0:C0 + CW], in_=cc, func=ACTF.Abs,
                accum_out=osum_t[:, s:s + 1])
            nc.vector.scalar_tensor_tensor(
                out=a3[:, 0:1, :], in0=c3[:, 0:1, :], scalar=-1.0,
                in1=c3[:, 0:1, :], op0=ALU.mult, op1=ALU.max)
            nc.vector.scalar_tensor_tensor(
                out=a3[:, FH - 1:FH, :], in0=c3[:, FH - 1:FH, :], scalar=-1.0,
                in1=c3[:, FH - 1:FH, :], op0=ALU.mult, op1=ALU.max)

            # ts = (a > thr), whole tile in one 4x op (no accum!)
            ts_t = mida.tile([128, FW], BF16, tag="ts", name=f"ts{s}")
            ts3 = ts_t.rearrange("p (a b) -> p a b", b=W)
            nc.vector.tensor_scalar(
                out=ts_t, in0=a_t, scalar1=thr_t, scalar2=None,
                op0=ALU.is_gt)

            # sum(ts) over center: ones-matmul accumulating across supertiles
            for k in range(9):
                nc.tensor.matmul(
                    out=opt_t, lhsT=o12,
                    rhs=ts_t[:, C0 + 512 * k:C0 + 512 * (k + 1)],
                    start=(s == 0 and k == 0), stop=(s == NS - 1 and k == 8),
                    skip_group_check=True)

            # d = W-derivative of ts (symmetric boundary), cols permuted:
            # cols 0..189 = d[w=1..190], col 190 = d[w=0], col 191 = d[w=191]
            d_t = midd.tile([128, FW], BF16, tag="d", name=f"d{s}")
            d3 = d_t.rearrange("p (a b) -> p a b", b=W)
            nc.vector.tensor_tensor(
                out=d3[:, :, 0:190], in0=ts3[:, :, 2:192],
                in1=ts3[:, :, 0:190], op=ALU.subtract)
            nc.vector.tensor_tensor(
                out=d3[:, :, 190:191], in0=ts3[:, :, 1:2],
                in1=ts3[:, :, 0:1], op=ALU.subtract)
            nc.vector.tensor_tensor(
                out=d3[:, :, 191:192], in0=ts3[:, :, 191:192],
                in1=ts3[:, :, 190:191], op=ALU.subtract)
            state[s] = (c_t, d_t)
            ctiles.append(c_t)

        def back(s):
            c_t, d_t = state.pop(s)
            cc = c_t[:, C0:C0 + CW]
            o12 = ones_t[:, 12 * s:12 * (s + 1)]

            # counts: materialize indicator at 4x, two exact bf16 pair-folds
            # at 2x, then a 3-matmul ones-reduce on PE (all correctly priced
            # by the scheduler's cost model, unlike fused DVE accum at 1x)
            def count_reduce(tag, op_psum, scalar1):
                ind = scr.tile([128, CW], BF16, tag="dum", name=f"{tag}i{s}")
                nc.vector.tensor_scalar(
                    out=ind, in0=cc, scalar1=scalar1, scalar2=None,
                    op0=ALU.is_lt)
                f1 = scr.tile([128, CW // 2], BF16, tag="f1", name=f"{tag}f{s}")
                nc.vector.tensor_tensor(
                    out=f1, in0=ind[:, 0:CW // 2], in1=ind[:, CW // 2:CW],
                    op=ALU.add)
                f2 = scr.tile([128, CW // 4], BF16, tag="f2", name=f"{tag}g{s}")
                nc.vector.tensor_tensor(
                    out=f2, in0=f1[:, 0:CW // 4], in1=f1[:, CW // 4:CW // 2],
                    op=ALU.add)
                for kk, (o0, w) in enumerate([(0, 512), (512, 512),
                                              (1024, 128)]):
                    nc.tensor.matmul(
                        out=op_psum[:, 0:w], lhsT=o12,
                        rhs=f2[:, o0:o0 + w],
                        start=(s == 0 and kk == 0),
                        stop=(s == NS - 1 and kk == 2),
                        skip_group_check=True)

            count_reduce("m", opm_t, 0.0)
            if s in ACT_TSM:
                dtsm = scr.tile([128, CW], BF16, tag="dum", name=f"dtsm{s}")
                nc.scalar.activation(
                    out=dtsm, in_=cc, func=ACTF.Sigmoid,
                    scale=-1.0e8, bias=bsig_t,
                    accum_out=tsmsum[:, s:s + 1])
                if s == NS - 1:
                    # close the opq accumulation group on the last supertile
                    nc.tensor.matmul(
                        out=opq_t[:, 0:128], lhsT=o12,
                        rhs=zero128_t, start=False, stop=True,
                        skip_group_check=True)
            else:
                count_reduce("q", opq_t, negthr_t)

            # grad = S_z(S_h(d)) via 3 H-shifted banded matmuls into PSUM
            for j in range(6):
                g_t = gps.tile([128, GRP], F32, tag="g", name=f"g{s}_{j}")
                base = C0 + GRP * j
                for di, (lhs, doff) in enumerate(
                        [(bz1_t, -W), (bz1_t, W), (bz2_t, 0)]):
                    for k, (o0, w) in enumerate([(0, 512), (512, 256)]):
                        off = base + o0 + doff
                        nc.tensor.matmul(
                            out=g_t[:, o0:o0 + w],
                            lhsT=lhs, rhs=d_t[:, off:off + w],
                            start=(di == 0), stop=(di == 2))
                # edge = (grad > 0): integer grad, sigmoid saturates
                e_t = scr.tile([128, GRP], BF16, tag="edge", name=f"e{s}_{j}")
                nc.scalar.activation(
                    out=e_t, in_=g_t, func=ACTF.Sigmoid,
                    scale=100.0, bias=nbias_t,
                    accum_out=edgesum[:, 6 * s + j:6 * s + j + 1])

        for s in range(NS):
            front(s)
            if s >= 1:
                back(s - 1)
        back(NS - 1)

        nc.scalar.copy(out=osb[:, 0:512], in_=opc_t)
        nc.scalar.copy(out=osb[:, 512:1024], in_=opt_t)
        nc.scalar.copy(out=osb[:, 1024:1536], in_=opm_t)
        nc.scalar.copy(out=osb[:, 1536:2048], in_=opq_t)

        nc.default_dma_engine.dma_start(out=part_d[:, 0:6], in_=osum_t)
        nc.default_dma_engine.dma_start(out=part_d[:, 6:12], in_=tsmsum)
        nc.default_dma_engine.dma_start(out=part_d[:, 12:48], in_=edgesum)
        nc.default_dma_engine.dma_start(out=osum_d, in_=osb)

    nc.compile()
    return nc


def _get_program():
    if "nc" not in _CACHE:
        _CACHE["nc"] = _build_program()
    return _CACHE["nc"]


def _make_in_maps(output, masks, loss_threshold):
    o = np.asarray(output, dtype=np.float32)
    m = np.asarray(masks, dtype=np.float32)
    c = np.copysign(o, np.float32(0.5) - m).astype(ml_dtypes.bfloat16)
    c5 = c.reshape(NV, Z, H, W)
    thr = np.full((1, 1), np.float32(np.asarray(loss_threshold)), np.float32)
    bz1, bz2, ones12 = _consts()
    in_maps = []
    for cid in range(NCORES):
        h0 = HC * cid
        idx = np.clip(np.arange(h0 - 1, h0 + HC + 1), 0, H - 1)
        c_sh = np.ascontiguousarray(c5[:, :, idx, :]).reshape(NV * Z, FW)
        in_maps.append({
            "c": c_sh, "thr": thr,
            "bz1": bz1, "bz2": bz2, "ones12": ones12,
        })
    return in_maps


def _combine(results):
    """Host-side tiny reduction: per-core partials -> loss scalar."""
    sum_ts = np.zeros(NV)
    sum_tsm = np.zeros(NV)
    sum_m = np.zeros(NV)
    sum_o = np.zeros(NV)
    sum_edge = np.zeros(NV)
    sum_c = np.zeros(NV)
    for r in results:
        p = np.asarray(r["partials"], dtype=np.float64)
        osum = np.asarray(r["osum"], dtype=np.float64)
        # [partition, s]: volume = 2s + partition//64, z = partition%64
        sum_o += p[:, 0:6].reshape(2, 64, NS).sum(1).T.reshape(-1)
        sum_tsm += p[:, 6:12].reshape(2, 64, NS).sum(1).T.reshape(-1)
        sum_edge += (p[:, 12:48].reshape(2, 64, NS, 6).sum(axis=(1, 3))
                     .T.reshape(-1))
        # osum rows are volumes directly (block ones lhsT)
        sum_c += osum[:, 0:512].sum(-1)
        sum_ts += osum[:, 512:1024].sum(-1)
        sum_m += osum[:, 1024:1536].sum(-1)
        sum_tsm += osum[:, 1536:2048].sum(-1)

    sum_om = 0.5 * (sum_o - sum_c)
    sum_eq = VOX - sum_ts - sum_m + 2.0 * sum_tsm

    freq = (sum_m / VOX).reshape(B, C)
    med = np.median(freq, axis=1, keepdims=True)
    w0 = 2.0 * med / (freq.min(axis=1, keepdims=True) + 1e-5)
    cw = (med / (freq + 1e-5)) * sum_eq.reshape(B, C) \
        + w0 * sum_edge.reshape(B, C)
    ps1 = sum_om.reshape(B, C)
    ps2 = (sum_o + sum_m).reshape(B, C)
    nom = (cw * ps1).sum(1)
    denom = (cw * ps2 + 1e-7).sum(1)
    loss = (1.0 - 2.0 * nom / denom).sum() / B
    return np.array([loss], dtype=np.float32)


def run(output, masks, loss_threshold, trace=False, **trace_kwargs):
    nc = _get_program()
    in_maps = _make_in_maps(output, masks, loss_threshold)
    res = run_bass_kernel_spmd(nc, in_maps, list(range(NCORES)),
                               trace=trace, **trace_kwargs)
    return _combine(res.results), res


def kernel(output, masks, loss_threshold):
    loss, _ = run(output, masks, loss_threshold)
    return loss


# revision 23
# speedup vs baseline: 1.1193x; 1.0607x over previous
"""Trainium2 Bass kernel for LogWeightedDICELossMultiClass3D.

Input: output (4,3,64,192,192) f32, masks (same), loss_threshold scalar.

Strategy: host packs both inputs into ONE bf16 tensor
    c = copysign(output, 0.5 - masks)        (sign bit = mask, |c| = output)
so DMA traffic is 2 bytes/voxel instead of 8. H=192 is sharded into 8
slabs of 24 rows (one per core, 1-row halo clamped on host).

Per core, 6 supertiles of 128 partitions (= 2 volumes x 64 z), free dim =
26 H-rows x 192 W. Reductions are expensive on this HW (any DVE op with
accum_out drops to 1x; ACT is 1 elem/cycle; PE ones-matmul is 1 col/cycle)
so each of the six per-(vol,z)/per-volume sums rides the cheapest slot:
  sum_o    : fused accum of the ACT Abs pass that materializes a=|c|
  sum_c    : PE ones-matmul on c  (sum_om = (sum_o - sum_c)/2 on host)
  sum_ts   : PE ones-matmul on ts (both accumulate over all supertiles
             into one [12,512] PSUM tile via a block ones lhsT)
  sum_m    : DVE is_lt + add-accum (1x)
  sum_tsm  : DVE is_lt + add-accum for 3 supertiles, ACT saturated
             sigmoid for 3 (engine balance)
  sum_edge : ts = (a > thr) on DVE (4x); d = W-deriv(ts) on DVE (2x);
             H-smooth (3 shifted taps) x Z-smooth (block-banded lhsT) on
             PE into PSUM; edge = (grad > 0) via saturated Sigmoid on
             ACT + accum (grad is integer-valued so this is exact)
Host combines the tiny partials into the scalar loss
(sum_eq = vox - sum_ts - sum_m + 2*sum_tsm).
"""

import numpy as np
import ml_dtypes

import concourse.bacc as bacc
import concourse.bass as bass
import concourse.tile as tile
from concourse import mybir
from concourse.bass_utils import run_bass_kernel_spmd

F32 = mybir.dt.float32
BF16 = mybir.dt.bfloat16
ALU = mybir.AluOpType
ACTF = mybir.ActivationFunctionType

B, C, Z, H, W = 4, 3, 64, 192, 192
NV = B * C            # 12 volumes
NCORES = 8
HC = H // NCORES      # 24 H-rows per core
NS = NV // 2          # 6 supertiles (2 volumes each)
FH = HC + 2           # 26 rows incl halo
FW = FH * W           # 4992 free elements per partition of c / a / ts / d
CW = HC * W           # 4608 center free elements
C0 = W                # flat offset of center region (row 1)
GRP = 768             # tap-group width (2 PSUM banks)
VOX = Z * H * W
ACT_TSM = (4, 5)      # supertiles whose tsm-count runs on ACT, not DVE

_CACHE = {}


def _band64():
    """[1,2,1] Z-smoothing matrix with scipy 'reflect' ends."""
    M = np.zeros((Z, Z), dtype=np.float64)
    for i in range(Z):
        M[i, i] = 2.0
        M[i, max(i - 1, 0)] += 1.0
        M[i, min(i + 1, Z - 1)] += 1.0
    return M


def _consts():
    Bz = _band64()
    blk = np.zeros((128, 128), dtype=np.float64)
    blk[:64, :64] = Bz
    blk[64:, 64:] = Bz
    bz1 = blk.astype(ml_dtypes.bfloat16)          # weights 1,2,3 - exact
    bz2 = (2.0 * blk).astype(ml_dtypes.bfloat16)  # weights 2,4,6 - exact
    # per-supertile [128,12] ones blocks: slice s has col 2s = ones on
    # partitions 0..63 (volume 2s) and col 2s+1 = ones on 64..127.
    ones12 = np.zeros((128, 12 * NS), dtype=ml_dtypes.bfloat16)
    for s in range(NS):
        ones12[:64, 12 * s + 2 * s] = 1.0
        ones12[64:, 12 * s + 2 * s + 1] = 1.0
    return bz1, bz2, ones12


def _build_program():
    nc = bacc.Bacc("TRN2", target_bir_lowering=False, debug=False,
                   num_devices=NCORES)
    c_d = nc.dram_tensor("c", [NV * Z, FW], BF16, kind="ExternalInput").ap()
    thr_d = nc.dram_tensor("thr", [1, 1], F32, kind="ExternalInput").ap()
    bz1_d = nc.dram_tensor("bz1", [128, 128], BF16, kind="ExternalInput").ap()
    bz2_d = nc.dram_tensor("bz2", [128, 128], BF16, kind="ExternalInput").ap()
    ones_d = nc.dram_tensor("ones12", [128, 12 * NS], BF16,
                            kind="ExternalInput").ap()
    part_d = nc.dram_tensor("partials", [128, 48], F32,
                            kind="ExternalOutput").ap()
    osum_d = nc.dram_tensor("osum", [12, 2048], F32,
                            kind="ExternalOutput").ap()

    from contextlib import ExitStack
    with tile.TileContext(nc) as tc, ExitStack() as ctx:
        consts = ctx.enter_context(tc.tile_pool(name="consts", bufs=1))
        io = ctx.enter_context(tc.tile_pool(name="io", bufs=4))
        mida = ctx.enter_context(tc.tile_pool(name="mida", bufs=2))
        midd = ctx.enter_context(tc.tile_pool(name="midd", bufs=3))
        scr = ctx.enter_context(tc.tile_pool(name="scr", bufs=3))
        slots = ctx.enter_context(tc.tile_pool(name="slots", bufs=1))
        gps = ctx.enter_context(tc.tile_pool(name="gps", bufs=2, space="PSUM"))
        ops = ctx.enter_context(tc.tile_pool(name="ops", bufs=1, space="PSUM"))

        thr_t = consts.tile([128, 1], F32)
        nc.gpsimd.dma_start(out=thr_t, in_=thr_d.to_broadcast([128, 1]))
        bz1_t = consts.tile([128, 128], BF16)
        nc.default_dma_engine.dma_start(out=bz1_t, in_=bz1_d)
        bz2_t = consts.tile([128, 128], BF16)
        nc.default_dma_engine.dma_start(out=bz2_t, in_=bz2_d)
        ones_t = consts.tile([128, 12 * NS], BF16)
        nc.default_dma_engine.dma_start(out=ones_t, in_=ones_d)
        nbias_t = consts.tile([128, 1], F32)
        nc.vector.memset(nbias_t, -50.0)
        negthr_t = consts.tile([128, 1], F32)
        nc.vector.tensor_scalar(out=negthr_t, in0=thr_t, scalar1=-1.0,
                                scalar2=None, op0=ALU.mult)
        # bias for ACT-side tsm count: sigmoid(-1e8*c - 1e8*thr)
        bsig_t = consts.tile([128, 1], F32)
        nc.vector.tensor_scalar(out=bsig_t, in0=thr_t, scalar1=-1.0e8,
                                scalar2=None, op0=ALU.mult)

        tsmsum = slots.tile([128, NS], F32)
        osum_t = slots.tile([128, NS], F32)
        zero128_t = slots.tile([128, 128], BF16)
        nc.vector.memset(zero128_t, 0.0)
        nc.vector.memset(tsmsum, 0.0)
        edgesum = slots.tile([128, 6 * NS], F32)
        osb = slots.tile([12, 2048], F32)

        # one PSUM bank each, accumulated across all supertiles
        opc_t = ops.tile([12, 512], F32, name="opc")
        opt_t = ops.tile([12, 512], F32, name="opt")
        opm_t = ops.tile([12, 512], F32, name="opm")
        opq_t = ops.tile([12, 512], F32, name="opq")

        state = {}
        ctiles = []

        def front(s):
            c_t = io.tile([128, FW], BF16, tag="c", name=f"c{s}")
            nc.default_dma_engine.dma_start(
                out=c_t, in_=c_d[128 * s:128 * (s + 1), :])
            cc = c_t[:, C0:C0 + CW]
            c3 = c_t.rearrange("p (a b) -> p a b", b=W)
            o12 = ones_t[:, 12 * s:12 * (s + 1)]

            # sum(c) over center: ones-matmul accumulating across supertiles
            for k in range(9):
                nc.tensor.matmul(
                    out=opc_t, lhsT=o12,
                    rhs=c_t[:, C0 + 512 * k:C0 + 512 * (k + 1)],
                    start=(s == 0 and k == 0), stop=(s == NS - 1 and k == 8),
                    skip_group_check=True)

            # a = |c|: ACT Abs on center with fused sum(o); halo rows on DVE
            a_t = mida.tile([128, FW], BF16, tag="a", name=f"a{s}")
            a3 = a_t.rearrange("p (a b) -> p a b", b=W)
            nc.scalar.activation(
                out=a_t[:, C0:C0 + CW], in_=cc, func=ACTF.Abs,
                accum_out=osum_t[:, s:s + 1])
            nc.vector.scalar_tensor_tensor(
                out=a3[:, 0:1, :], in0=c3[:, 0:1, :], scalar=-1.0,
                in1=c3[:, 0:1, :], op0=ALU.mult, op1=ALU.max)
            nc.vector.scalar_tensor_tensor(
                out=a3[:, FH - 1:FH, :], in0=c3[:, FH - 1:FH, :], scalar=-1.0,
                in1=c3[:, FH - 1:FH, :], op0=ALU.mult, op1=ALU.max)

            # ts = (a > thr), whole tile in one 4x op (no accum!)
            ts_t = mida.tile([128, FW], BF16, tag="ts", name=f"ts{s}")
            ts3 = ts_t.rearrange("p (a b) -> p a b", b=W)
            nc.vector.tensor_scalar(
                out=ts_t, in0=a_t, scalar1=thr_t, scalar2=None,
                op0=ALU.is_gt)

            # sum(ts) over center: ones-matmul accumulating across supertiles
            for k in range(9):
                nc.tensor.matmul(
                    out=opt_t, lhsT=o12,
                    rhs=ts_t[:, C0 + 512 * k:C0 + 512 * (k + 1)],
                    start=(s == 0 and k == 0), stop=(s == NS - 1 and k == 8),
                    skip_group_check=True)

            # d = W-derivative of ts (symmetric boundary), cols permuted:
            # cols 0..189 = d[w=1..190], col 190 = d[w=0], col 191 = d[w=191]
            d_t = midd.tile([128, FW], BF16, tag="d", name=f"d{s}")
            d3 = d_t.rearrange("p (a b) -> p a b", b=W)
            nc.vector.tensor_tensor(
                out=d3[:, :, 0:190], in0=ts3[:, :, 2:192],
                in1=ts3[:, :, 0:190], op=ALU.subtract)
            nc.vector.tensor_tensor(
                out=d3[:, :, 190:191], in0=ts3[:, :, 1:2],
                in1=ts3[:, :, 0:1], op=ALU.subtract)
            nc.vector.tensor_tensor(
                out=d3[:, :, 191:192], in0=ts3[:, :, 191:192],
                in1=ts3[:, :, 190:191], op=ALU.subtract)
            state[s] = (c_t, d_t)
            ctiles.append(c_t)

        def back(s):
            c_t, d_t = state.pop(s)
            cc = c_t[:, C0:C0 + CW]
            o12 = ones_t[:, 12 * s:12 * (s + 1)]

            # counts: materialize indicator at 4x, two exact bf16 pair-folds
            # at 2x, then a 3-matmul ones-reduce on PE (all correctly priced
            # by the scheduler's cost model, unlike fused DVE accum at 1x)
            def count_reduce(tag, op_psum, scalar1):
                ind = scr.tile([128, CW], BF16, tag="dum", name=f"{tag}i{s}")
                nc.vector.tensor_scalar(
                    out=ind, in0=cc, scalar1=scalar1, scalar2=None,
                    op0=ALU.is_lt)
                f1 = scr.tile([128, CW // 2], BF16, tag="f1", name=f"{tag}f{s}")
                nc.vector.tensor_tensor(
                    out=f1, in0=ind[:, 0:CW // 2], in1=ind[:, CW // 2:CW],
                    op=ALU.add)
                f2 = scr.tile([128, CW // 4], BF16, tag="f2", name=f"{tag}g{s}")
                nc.vector.tensor_tensor(
                    out=f2, in0=f1[:, 0:CW // 4], in1=f1[:, CW // 4:CW // 2],
                    op=ALU.add)
                for kk, (o0, w) in enumerate([(0, 512), (512, 512),
                                              (1024, 128)]):
                    nc.tensor.matmul(
                        out=op_psum[:, 0:w], lhsT=o12,
                        rhs=f2[:, o0:o0 + w],
                        start=(s == 0 and kk == 0),
                        stop=(s == NS - 1 and kk == 2),
                        skip_group_check=True)

            count_reduce("m", opm_t, 0.0)
            if s in ACT_TSM:
                dtsm = scr.tile([128, CW], BF16, tag="dum", name=f"dtsm{s}")
                nc.scalar.activation(
                    out=dtsm, in_=cc, func=ACTF.Sigmoid,
                    scale=-1.0e8, bias=bsig_t,
                    accum_out=tsmsum[:, s:s + 1])
                if s == NS - 1:
                    # close the opq accumulation group on the last supertile
                    nc.tensor.matmul(
                        out=opq_t[:, 0:128], lhsT=o12,
                        rhs=zero128_t, start=False, stop=True,
                        skip_group_check=True)
            else:
                count_reduce("q", opq_t, negthr_t)

            # grad = S_z(S_h(d)) via 3 H-shifted banded matmuls into PSUM
            for j in range(6):
                g_t = gps.tile([128, GRP], F32, tag="g", name=f"g{s}_{j}")
                base = C0 + GRP * j
                for di, (lhs, doff) in enumerate(
                        [(bz1_t, -W), (bz1_t, W), (bz2_t, 0)]):
                    for k, (o0, w) in enumerate([(0, 512), (512, 256)]):
                        off = base + o0 + doff
                        nc.tensor.matmul(
                            out=g_t[:, o0:o0 + w],
                            lhsT=lhs, rhs=d_t[:, off:off + w],
                            start=(di == 0), stop=(di == 2))
                # edge = (grad > 0): integer grad, sigmoid saturates
                e_t = scr.tile([128, GRP], BF16, tag="edge", name=f"e{s}_{j}")
                nc.scalar.activation(
                    out=e_t, in_=g_t, func=ACTF.Sigmoid,
                    scale=100.0, bias=nbias_t,
                    accum_out=edgesum[:, 6 * s + j:6 * s + j + 1])

        for s in range(NS):
            front(s)
            if s >= 1:
                back(s - 1)
        back(NS - 1)

        nc.scalar.copy(out=osb[:, 0:512], in_=opc_t)
        nc.scalar.copy(out=osb[:, 512:1024], in_=opt_t)
        nc.scalar.copy(out=osb[:, 1024:1536], in_=opm_t)
        nc.scalar.copy(out=osb[:, 1536:2048], in_=opq_t)

        nc.default_dma_engine.dma_start(out=part_d[:, 0:6], in_=osum_t)
        nc.default_dma_engine.dma_start(out=part_d[:, 6:12], in_=tsmsum)
        nc.default_dma_engine.dma_start(out=part_d[:, 12:48], in_=edgesum)
        nc.default_dma_engine.dma_start(out=osum_d, in_=osb)

    nc.compile()
    return nc


def _get_program():
    if "nc" not in _CACHE:
        _CACHE["nc"] = _build_program()
    return _CACHE["nc"]


def _make_in_maps(output, masks, loss_threshold):
    o = np.asarray(output, dtype=np.float32)
    m = np.asarray(masks, dtype=np.float32)
    c = np.copysign(o, np.float32(0.5) - m).astype(ml_dtypes.bfloat16)
    c5 = c.reshape(NV, Z, H, W)
    thr = np.full((1, 1), np.float32(np.asarray(loss_threshold)), np.float32)
    bz1, bz2, ones12 = _consts()
    in_maps = []
    for cid in range(NCORES):
        h0 = HC * cid
        idx = np.clip(np.arange(h0 - 1, h0 + HC + 1), 0, H - 1)
        c_sh = np.ascontiguousarray(c5[:, :, idx, :]).reshape(NV * Z, FW)
        in_maps.append({
            "c": c_sh, "thr": thr,
            "bz1": bz1, "bz2": bz2, "ones12": ones12,
        })
    return in_maps


def _combine(results):
    """Host-side tiny reduction: per-core partials -> loss scalar."""
    sum_ts = np.zeros(NV)
    sum_tsm = np.zeros(NV)
    sum_m = np.zeros(NV)
    sum_o = np.zeros(NV)
    sum_edge = np.zeros(NV)
    sum_c = np.zeros(NV)
    for r in results:
        p = np.asarray(r["partials"], dtype=np.float64)
        osum = np.asarray(r["osum"], dtype=np.float64)
        # [partition, s]: volume = 2s + partition//64, z = partition%64
        sum_o += p[:, 0:6].reshape(2, 64, NS).sum(1).T.reshape(-1)
        sum_tsm += p[:, 6:12].reshape(2, 64, NS).sum(1).T.reshape(-1)
        sum_edge += (p[:, 12:48].reshape(2, 64, NS, 6).sum(axis=(1, 3))
                     .T.reshape(-1))
        # osum rows are volumes directly (block ones lhsT)
        sum_c += osum[:, 0:512].sum(-1)
        sum_ts += osum[:, 512:1024].sum(-1)
        sum_m += osum[:, 1024:1536].sum(-1)
        sum_tsm += osum[:, 1536:2048].sum(-1)

    sum_om = 0.5 * (sum_o - sum_c)
    sum_eq = VOX - sum_ts - sum_m + 2.0 * sum_tsm

    freq = (sum_m / VOX).reshape(B, C)
    med = np.median(freq, axis=1, keepdims=True)
    w0 = 2.0 * med / (freq.min(axis=1, keepdims=True) + 1e-5)
    cw = (med / (freq + 1e-5)) * sum_eq.reshape(B, C) \
        + w0 * sum_edge.reshape(B, C)
    ps1 = sum_om.reshape(B, C)
    ps2 = (sum_o + sum_m).reshape(B, C)
    nom = (cw * ps1).sum(1)
    denom = (cw * ps2 + 1e-7).sum(1)
    loss = (1.0 - 2.0 * nom / denom).sum() / B
    return np.array([loss], dtype=np.float32)


def run(output, masks, loss_threshold, trace=False, **trace_kwargs):
    nc = _get_program()
    in_maps = _make_in_maps(output, masks, loss_threshold)
    res = run_bass_kernel_spmd(nc, in_maps, list(range(NCORES)),
                               trace=trace, **trace_kwargs)
    return _combine(res.results), res


def kernel(output, masks, loss_threshold):
    loss, _ = run(output, masks, loss_threshold)
    return loss


# revision 24
# speedup vs baseline: 1.1365x; 1.0154x over previous
"""Trainium2 Bass kernel for LogWeightedDICELossMultiClass3D.

Input: output (4,3,64,192,192) f32, masks (same), loss_threshold scalar.

Strategy: host packs both inputs into ONE bf16 tensor
    c = copysign(output, 0.5 - masks)        (sign bit = mask, |c| = output)
so DMA traffic is 2 bytes/voxel instead of 8. H=192 is sharded into 8
slabs of 24 rows (one per core, 1-row halo clamped on host).

Per core, 6 supertiles of 128 partitions (= 2 volumes x 64 z), free dim =
26 H-rows x 192 W. Reductions are expensive on this HW (any DVE op with
accum_out drops to 1x; ACT is 1 elem/cycle; PE ones-matmul is 1 col/cycle)
so each of the six per-(vol,z)/per-volume sums rides the cheapest slot:
  sum_o    : fused accum of the ACT Abs pass that materializes a=|c|
  sum_c    : PE ones-matmul on c  (sum_om = (sum_o - sum_c)/2 on host)
  sum_ts   : PE ones-matmul on ts (both accumulate over all supertiles
             into one [12,512] PSUM tile via a block ones lhsT)
  sum_m    : DVE is_lt + add-accum (1x)
  sum_tsm  : DVE is_lt + add-accum for 3 supertiles, ACT saturated
             sigmoid for 3 (engine balance)
  sum_edge : ts = (a > thr) on DVE (4x); d = W-deriv(ts) on DVE (2x);
             H-smooth (3 shifted taps) x Z-smooth (block-banded lhsT) on
             PE into PSUM; edge = (grad > 0) via saturated Sigmoid on
             ACT + accum (grad is integer-valued so this is exact)
Host combines the tiny partials into the scalar loss
(sum_eq = vox - sum_ts - sum_m + 2*sum_tsm).
"""

import numpy as np
import ml_dtypes

import concourse.bacc as bacc
import concourse.bass as bass
import concourse.tile as tile
from concourse import mybir
from concourse.bass_utils import run_bass_kernel_spmd

F32 = mybir.dt.float32
BF16 = mybir.dt.bfloat16
ALU = mybir.AluOpType
ACTF = mybir.ActivationFunctionType

B, C, Z, H, W = 4, 3, 64, 192, 192
NV = B * C            # 12 volumes
NCORES = 8
HC = H // NCORES      # 24 H-rows per core
NS = NV // 2          # 6 supertiles (2 volumes each)
FH = HC + 2           # 26 rows incl halo
FW = FH * W           # 4992 free elements per partition of c / a / ts / d
CW = HC * W           # 4608 center free elements
C0 = W                # flat offset of center region (row 1)
GRP = 768             # tap-group width (2 PSUM banks)
VOX = Z * H * W
ACT_TSM = (4, 5)      # supertiles whose tsm-count runs on ACT, not DVE

_CACHE = {}


def _band64():
    """[1,2,1] Z-smoothing matrix with scipy 'reflect' ends."""
    M = np.zeros((Z, Z), dtype=np.float64)
    for i in range(Z):
        M[i, i] = 2.0
        M[i, max(i - 1, 0)] += 1.0
        M[i, min(i + 1, Z - 1)] += 1.0
    return M


def _consts():
    Bz = _band64()
    blk = np.zeros((128, 128), dtype=np.float64)
    blk[:64, :64] = Bz
    blk[64:, 64:] = Bz
    bz1 = blk.astype(ml_dtypes.bfloat16)          # weights 1,2,3 - exact
    bz2 = (2.0 * blk).astype(ml_dtypes.bfloat16)  # weights 2,4,6 - exact
    # per-supertile [128,12] ones blocks: slice s has col 2s = ones on
    # partitions 0..63 (volume 2s) and col 2s+1 = ones on 64..127.
    ones12 = np.zeros((128, 12 * NS), dtype=ml_dtypes.bfloat16)
    for s in range(NS):
        ones12[:64, 12 * s + 2 * s] = 1.0
        ones12[64:, 12 * s + 2 * s + 1] = 1.0
    return bz1, bz2, ones12


def _build_program():
    nc = bacc.Bacc("TRN2", target_bir_lowering=False, debug=False,
                   num_devices=NCORES)
    c_d = nc.dram_tensor("c", [NV * Z, FW], BF16, kind="ExternalInput").ap()
    thr_d = nc.dram_tensor("thr", [1, 1], F32, kind="ExternalInput").ap()
    bz1_d = nc.dram_tensor("bz1", [128, 128], BF16, kind="ExternalInput").ap()
    bz2_d = nc.dram_tensor("bz2", [128, 128], BF16, kind="ExternalInput").ap()
    ones_d = nc.dram_tensor("ones12", [128, 12 * NS], BF16,
                            kind="ExternalInput").ap()
    part_d = nc.dram_tensor("partials", [128, 48], F32,
                            kind="ExternalOutput").ap()
    osum_d = nc.dram_tensor("osum", [12, 2048], F32,
                            kind="ExternalOutput").ap()

    from contextlib import ExitStack
    with tile.TileContext(nc) as tc, ExitStack() as ctx:
        consts = ctx.enter_context(tc.tile_pool(name="consts", bufs=1))
        io = ctx.enter_context(tc.tile_pool(name="io", bufs=4))
        mida = ctx.enter_context(tc.tile_pool(name="mida", bufs=3))
        midd = ctx.enter_context(tc.tile_pool(name="midd", bufs=3))
        scr = ctx.enter_context(tc.tile_pool(name="scr", bufs=3))
        slots = ctx.enter_context(tc.tile_pool(name="slots", bufs=1))
        gps = ctx.enter_context(tc.tile_pool(name="gps", bufs=2, space="PSUM"))
        ops = ctx.enter_context(tc.tile_pool(name="ops", bufs=1, space="PSUM"))

        thr_t = consts.tile([128, 1], F32)
        nc.gpsimd.dma_start(out=thr_t, in_=thr_d.to_broadcast([128, 1]))
        bz1_t = consts.tile([128, 128], BF16)
        nc.default_dma_engine.dma_start(out=bz1_t, in_=bz1_d)
        bz2_t = consts.tile([128, 128], BF16)
        nc.default_dma_engine.dma_start(out=bz2_t, in_=bz2_d)
        ones_t = consts.tile([128, 12 * NS], BF16)
        nc.default_dma_engine.dma_start(out=ones_t, in_=ones_d)
        nbias_t = consts.tile([128, 1], F32)
        nc.vector.memset(nbias_t, -50.0)
        negthr_t = consts.tile([128, 1], F32)
        nc.vector.tensor_scalar(out=negthr_t, in0=thr_t, scalar1=-1.0,
                                scalar2=None, op0=ALU.mult)
        # bias for ACT-side tsm count: sigmoid(-1e8*c - 1e8*thr)
        bsig_t = consts.tile([128, 1], F32)
        nc.vector.tensor_scalar(out=bsig_t, in0=thr_t, scalar1=-1.0e8,
                                scalar2=None, op0=ALU.mult)

        tsmsum = slots.tile([128, NS], F32)
        osum_t = slots.tile([128, NS], F32)
        zero128_t = slots.tile([128, 128], BF16)
        nc.vector.memset(zero128_t, 0.0)
        nc.vector.memset(tsmsum, 0.0)
        edgesum = slots.tile([128, 6 * NS], F32)
        osb = slots.tile([12, 2048], F32)

        # one PSUM bank each, accumulated across all supertiles
        opc_t = ops.tile([12, 512], F32, name="opc")
        opt_t = ops.tile([12, 512], F32, name="opt")
        opm_t = ops.tile([12, 512], F32, name="opm")
        opq_t = ops.tile([12, 512], F32, name="opq")

        state = {}
        ctiles = []

        def front(s):
            c_t = io.tile([128, FW], BF16, tag="c", name=f"c{s}")
            nc.default_dma_engine.dma_start(
                out=c_t, in_=c_d[128 * s:128 * (s + 1), :])
            cc = c_t[:, C0:C0 + CW]
            c3 = c_t.rearrange("p (a b) -> p a b", b=W)
            o12 = ones_t[:, 12 * s:12 * (s + 1)]

            # sum(c) over center: ones-matmul accumulating across supertiles
            for k in range(9):
                nc.tensor.matmul(
                    out=opc_t, lhsT=o12,
                    rhs=c_t[:, C0 + 512 * k:C0 + 512 * (k + 1)],
                    start=(s == 0 and k == 0), stop=(s == NS - 1 and k == 8),
                    skip_group_check=True)

            # a = |c|: ACT Abs on center with fused sum(o); halo rows on DVE
            a_t = mida.tile([128, FW], BF16, tag="a", name=f"a{s}")
            a3 = a_t.rearrange("p (a b) -> p a b", b=W)
            nc.scalar.activation(
                out=a_t[:, C0:C0 + CW], in_=cc, func=ACTF.Abs,
                accum_out=osum_t[:, s:s + 1])
            nc.vector.scalar_tensor_tensor(
                out=a3[:, 0:1, :], in0=c3[:, 0:1, :], scalar=-1.0,
                in1=c3[:, 0:1, :], op0=ALU.mult, op1=ALU.max)
            nc.vector.scalar_tensor_tensor(
                out=a3[:, FH - 1:FH, :], in0=c3[:, FH - 1:FH, :], scalar=-1.0,
                in1=c3[:, FH - 1:FH, :], op0=ALU.mult, op1=ALU.max)

            # ts = (a > thr), whole tile in one 4x op (no accum!)
            ts_t = mida.tile([128, FW], BF16, tag="ts", name=f"ts{s}")
            ts3 = ts_t.rearrange("p (a b) -> p a b", b=W)
            nc.vector.tensor_scalar(
                out=ts_t, in0=a_t, scalar1=thr_t, scalar2=None,
                op0=ALU.is_gt)

            # d = W-derivative of ts (symmetric boundary), cols permuted:
            # cols 0..189 = d[w=1..190], col 190 = d[w=0], col 191 = d[w=191]
            d_t = midd.tile([128, FW], BF16, tag="d", name=f"d{s}")
            d3 = d_t.rearrange("p (a b) -> p a b", b=W)
            nc.vector.tensor_tensor(
                out=d3[:, :, 0:190], in0=ts3[:, :, 2:192],
                in1=ts3[:, :, 0:190], op=ALU.subtract)
            nc.vector.tensor_tensor(
                out=d3[:, :, 190:191], in0=ts3[:, :, 1:2],
                in1=ts3[:, :, 0:1], op=ALU.subtract)
            nc.vector.tensor_tensor(
                out=d3[:, :, 191:192], in0=ts3[:, :, 191:192],
                in1=ts3[:, :, 190:191], op=ALU.subtract)
            state[s] = (c_t, ts_t, d_t)
            ctiles.append(c_t)

        def back(s):
            c_t, ts_t, d_t = state.pop(s)
            cc = c_t[:, C0:C0 + CW]
            o12 = ones_t[:, 12 * s:12 * (s + 1)]

            # counts: materialize indicator at 4x, two exact bf16 pair-folds
            # at 2x, then a 3-matmul ones-reduce on PE (all correctly priced
            # by the scheduler's cost model, unlike fused DVE accum at 1x)
            def count_reduce(tag, op_psum, scalar1):
                ind = scr.tile([128, CW], BF16, tag="dum", name=f"{tag}i{s}")
                nc.vector.tensor_scalar(
                    out=ind, in0=cc, scalar1=scalar1, scalar2=None,
                    op0=ALU.is_lt)
                f1 = scr.tile([128, CW // 2], BF16, tag="f1", name=f"{tag}f{s}")
                nc.vector.tensor_tensor(
                    out=f1, in0=ind[:, 0:CW // 2], in1=ind[:, CW // 2:CW],
                    op=ALU.add)
                f2 = scr.tile([128, CW // 4], BF16, tag="f2", name=f"{tag}g{s}")
                nc.vector.tensor_tensor(
                    out=f2, in0=f1[:, 0:CW // 4], in1=f1[:, CW // 4:CW // 2],
                    op=ALU.add)
                for kk, (o0, w) in enumerate([(0, 512), (512, 512),
                                              (1024, 128)]):
                    nc.tensor.matmul(
                        out=op_psum[:, 0:w], lhsT=o12,
                        rhs=f2[:, o0:o0 + w],
                        start=(s == 0 and kk == 0),
                        stop=(s == NS - 1 and kk == 2),
                        skip_group_check=True)

            # taps feed the ACT edge passes (the busiest engine): keep them
            # at the head of the PE queue for this supertile
            # grad = S_z(S_h(d)) via 3 H-shifted banded matmuls into PSUM
            for j in range(6):
                g_t = gps.tile([128, GRP], F32, tag="g", name=f"g{s}_{j}")
                base = C0 + GRP * j
                for di, (lhs, doff) in enumerate(
                        [(bz1_t, -W), (bz1_t, W), (bz2_t, 0)]):
                    for k, (o0, w) in enumerate([(0, 512), (512, 256)]):
                        off = base + o0 + doff
                        nc.tensor.matmul(
                            out=g_t[:, o0:o0 + w],
                            lhsT=lhs, rhs=d_t[:, off:off + w],
                            start=(di == 0), stop=(di == 2))
                # edge = (grad > 0): integer grad, sigmoid saturates
                e_t = scr.tile([128, GRP], BF16, tag="edge", name=f"e{s}_{j}")
                nc.scalar.activation(
                    out=e_t, in_=g_t, func=ACTF.Sigmoid,
                    scale=100.0, bias=nbias_t,
                    accum_out=edgesum[:, 6 * s + j:6 * s + j + 1])
            # sum(ts) over center: ones-matmul accumulating across supertiles
            for k in range(9):
                nc.tensor.matmul(
                    out=opt_t, lhsT=o12,
                    rhs=ts_t[:, C0 + 512 * k:C0 + 512 * (k + 1)],
                    start=(s == 0 and k == 0), stop=(s == NS - 1 and k == 8),
                    skip_group_check=True)

            count_reduce("m", opm_t, 0.0)
            if s in ACT_TSM:
                dtsm = scr.tile([128, CW], BF16, tag="dum", name=f"dtsm{s}")
                nc.scalar.activation(
                    out=dtsm, in_=cc, func=ACTF.Sigmoid,
                    scale=-1.0e8, bias=bsig_t,
                    accum_out=tsmsum[:, s:s + 1])
                if s == NS - 1:
                    # close the opq accumulation group on the last supertile
                    nc.tensor.matmul(
                        out=opq_t[:, 0:128], lhsT=o12,
                        rhs=zero128_t, start=False, stop=True,
                        skip_group_check=True)
            else:
                count_reduce("q", opq_t, negthr_t)

        for s in range(NS):
            front(s)
            if s >= 1:
                back(s - 1)
        back(NS - 1)

        nc.scalar.copy(out=osb[:, 0:512], in_=opc_t)
        nc.scalar.copy(out=osb[:, 512:1024], in_=opt_t)
        nc.scalar.copy(out=osb[:, 1024:1536], in_=opm_t)
        nc.scalar.copy(out=osb[:, 1536:2048], in_=opq_t)

        nc.default_dma_engine.dma_start(out=part_d[:, 0:6], in_=osum_t)
        nc.default_dma_engine.dma_start(out=part_d[:, 6:12], in_=tsmsum)
        nc.default_dma_engine.dma_start(out=part_d[:, 12:48], in_=edgesum)
        nc.default_dma_engine.dma_start(out=osum_d, in_=osb)

    nc.compile()
    return nc


def _get_program():
    if "nc" not in _CACHE:
        _CACHE["nc"] = _build_program()
    return _CACHE["nc"]


def _make_in_maps(output, masks, loss_threshold):
    o = np.asarray(output, dtype=np.float32)
    m = np.asarray(masks, dtype=np.float32)
    c = np.copysign(o, np.float32(0.5) - m).astype(ml_dtypes.bfloat16)
    c5 = c.reshape(NV, Z, H, W)
    thr = np.full((1, 1), np.float32(np.asarray(loss_threshold)), np.float32)
    bz1, bz2, ones12 = _consts()
    in_maps = []
    for cid in range(NCORES):
        h0 = HC * cid
        idx = np.clip(np.arange(h0 - 1, h0 + HC + 1), 0, H - 1)
        c_sh = np.ascontiguousarray(c5[:, :, idx, :]).reshape(NV * Z, FW)
        in_maps.append({
            "c": c_sh, "thr": thr,
            "bz1": bz1, "bz2": bz2, "ones12": ones12,
        })
    return in_maps


def _combine(results):
    """Host-side tiny reduction: per-core partials -> loss scalar."""
    sum_ts = np.zeros(NV)
    sum_tsm = np.zeros(NV)
    sum_m = np.zeros(NV)
    sum_o = np.zeros(NV)
    sum_edge = np.zeros(NV)
    sum_c = np.zeros(NV)
    for r in results:
        p = np.asarray(r["partials"], dtype=np.float64)
        osum = np.asarray(r["osum"], dtype=np.float64)
        # [partition, s]: volume = 2s + partition//64, z = partition%64
        sum_o += p[:, 0:6].reshape(2, 64, NS).sum(1).T.reshape(-1)
        sum_tsm += p[:, 6:12].reshape(2, 64, NS).sum(1).T.reshape(-1)
        sum_edge += (p[:, 12:48].reshape(2, 64, NS, 6).sum(axis=(1, 3))
                     .T.reshape(-1))
        # osum rows are volumes directly (block ones lhsT)
        sum_c += osum[:, 0:512].sum(-1)
        sum_ts += osum[:, 512:1024].sum(-1)
        sum_m += osum[:, 1024:1536].sum(-1)
        sum_tsm += osum[:, 1536:2048].sum(-1)

    sum_om = 0.5 * (sum_o - sum_c)
    sum_eq = VOX - sum_ts - sum_m + 2.0 * sum_tsm

    freq = (sum_m / VOX).reshape(B, C)
    med = np.median(freq, axis=1, keepdims=True)
    w0 = 2.0 * med / (freq.min(axis=1, keepdims=True) + 1e-5)
    cw = (med / (freq + 1e-5)) * sum_eq.reshape(B, C) \
        + w0 * sum_edge.reshape(B, C)
    ps1 = sum_om.reshape(B, C)
    ps2 = (sum_o + sum_m).reshape(B, C)
    nom = (cw * ps1).sum(1)
    denom = (cw * ps2 + 1e-7).sum(1)
    loss = (1.0 - 2.0 * nom / denom).sum() / B
    return np.array([loss], dtype=np.float32)


def run(output, masks, loss_threshold, trace=False, **trace_kwargs):
    nc = _get_program()
    in_maps = _make_in_maps(output, masks, loss_threshold)
    res = run_bass_kernel_spmd(nc, in_maps, list(range(NCORES)),
                               trace=trace, **trace_kwargs)
    return _combine(res.results), res


def kernel(output, masks, loss_threshold):
    loss, _ = run(output, masks, loss_threshold)
    return loss


# revision 26
# speedup vs baseline: 1.1457x; 1.0081x over previous
"""Trainium2 Bass kernel for LogWeightedDICELossMultiClass3D.

Input: output (4,3,64,192,192) f32, masks (same), loss_threshold scalar.

Strategy: host packs both inputs into ONE bf16 tensor
    c = copysign(output, 0.5 - masks)        (sign bit = mask, |c| = output)
so DMA traffic is 2 bytes/voxel instead of 8. H=192 is sharded into 8
slabs of 24 rows (one per core, 1-row halo clamped on host).

Per core, 6 supertiles of 128 partitions (= 2 volumes x 64 z), free dim =
26 H-rows x 192 W. Reductions are expensive on this HW (any DVE op with
accum_out drops to 1x; ACT is 1 elem/cycle; PE ones-matmul is 1 col/cycle)
so each of the six per-(vol,z)/per-volume sums rides the cheapest slot:
  sum_o    : fused accum of the ACT Abs pass that materializes a=|c|
  sum_c    : PE ones-matmul on c  (sum_om = (sum_o - sum_c)/2 on host)
  sum_ts   : PE ones-matmul on ts (both accumulate over all supertiles
             into one [12,512] PSUM tile via a block ones lhsT)
  sum_m    : DVE is_lt + add-accum (1x)
  sum_tsm  : DVE is_lt + add-accum for 3 supertiles, ACT saturated
             sigmoid for 3 (engine balance)
  sum_edge : ts = (a > thr) on DVE (4x); d = W-deriv(ts) on DVE (2x);
             H-smooth (3 shifted taps) x Z-smooth (block-banded lhsT) on
             PE into PSUM; edge = (grad > 0) via saturated Sigmoid on
             ACT + accum (grad is integer-valued so this is exact)
Host combines the tiny partials into the scalar loss
(sum_eq = vox - sum_ts - sum_m + 2*sum_tsm).
"""

import numpy as np
import ml_dtypes

import concourse.bacc as bacc
import concourse.bass as bass
import concourse.tile as tile
from concourse import mybir
from concourse.bass_utils import run_bass_kernel_spmd

F32 = mybir.dt.float32
BF16 = mybir.dt.bfloat16
ALU = mybir.AluOpType
ACTF = mybir.ActivationFunctionType

B, C, Z, H, W = 4, 3, 64, 192, 192
NV = B * C            # 12 volumes
NCORES = 8
HC = H // NCORES      # 24 H-rows per core
NS = NV // 2          # 6 supertiles (2 volumes each)
FH = HC + 2           # 26 rows incl halo
FW = FH * W           # 4992 free elements per partition of c / a / ts / d
CW = HC * W           # 4608 center free elements
C0 = W                # flat offset of center region (row 1)
GRP = 768             # tap-group width (2 PSUM banks)
VOX = Z * H * W
ACT_TSM = (4, 5)      # supertiles whose tsm-count runs on ACT, not DVE

_CACHE = {}


def _band64():
    """[1,2,1] Z-smoothing matrix with scipy 'reflect' ends."""
    M = np.zeros((Z, Z), dtype=np.float64)
    for i in range(Z):
        M[i, i] = 2.0
        M[i, max(i - 1, 0)] += 1.0
        M[i, min(i + 1, Z - 1)] += 1.0
    return M


def _consts():
    Bz = _band64()
    blk = np.zeros((128, 128), dtype=np.float64)
    blk[:64, :64] = Bz
    blk[64:, 64:] = Bz
    bz1 = blk.astype(ml_dtypes.bfloat16)          # weights 1,2,3 - exact
    bz2 = (2.0 * blk).astype(ml_dtypes.bfloat16)  # weights 2,4,6 - exact
    # per-supertile [128,12] ones blocks: slice s has col 2s = ones on
    # partitions 0..63 (volume 2s) and col 2s+1 = ones on 64..127.
    ones12 = np.zeros((128, 12 * NS), dtype=ml_dtypes.bfloat16)
    for s in range(NS):
        ones12[:64, 12 * s + 2 * s] = 1.0
        ones12[64:, 12 * s + 2 * s + 1] = 1.0
    return bz1, bz2, ones12


def _build_program():
    nc = bacc.Bacc("TRN2", target_bir_lowering=False, debug=False,
                   num_devices=NCORES)
    c_d = nc.dram_tensor("c", [NV * Z, FW], BF16, kind="ExternalInput").ap()
    thr_d = nc.dram_tensor("thr", [1, 1], F32, kind="ExternalInput").ap()
    bz1_d = nc.dram_tensor("bz1", [128, 128], BF16, kind="ExternalInput").ap()
    bz2_d = nc.dram_tensor("bz2", [128, 128], BF16, kind="ExternalInput").ap()
    ones_d = nc.dram_tensor("ones12", [128, 12 * NS], BF16,
                            kind="ExternalInput").ap()
    part_d = nc.dram_tensor("partials", [128, 48], F32,
                            kind="ExternalOutput").ap()
    osum_d = nc.dram_tensor("osum", [12, 2048], F32,
                            kind="ExternalOutput").ap()

    from contextlib import ExitStack
    with tile.TileContext(nc) as tc, ExitStack() as ctx:
        consts = ctx.enter_context(tc.tile_pool(name="consts", bufs=1))
        io = ctx.enter_context(tc.tile_pool(name="io", bufs=4))
        mida = ctx.enter_context(tc.tile_pool(name="mida", bufs=3))
        midd = ctx.enter_context(tc.tile_pool(name="midd", bufs=3))
        scr = ctx.enter_context(tc.tile_pool(name="scr", bufs=3))
        slots = ctx.enter_context(tc.tile_pool(name="slots", bufs=1))
        gps = ctx.enter_context(tc.tile_pool(name="gps", bufs=2, space="PSUM"))
        ops = ctx.enter_context(tc.tile_pool(name="ops", bufs=1, space="PSUM"))

        thr_t = consts.tile([128, 1], F32)
        nc.gpsimd.dma_start(out=thr_t, in_=thr_d.to_broadcast([128, 1]))
        bz1_t = consts.tile([128, 128], BF16)
        nc.default_dma_engine.dma_start(out=bz1_t, in_=bz1_d)
        bz2_t = consts.tile([128, 128], BF16)
        nc.default_dma_engine.dma_start(out=bz2_t, in_=bz2_d)
        ones_t = consts.tile([128, 12 * NS], BF16)
        nc.default_dma_engine.dma_start(out=ones_t, in_=ones_d)
        nbias_t = consts.tile([128, 1], F32)
        nc.vector.memset(nbias_t, -50.0)
        negthr_t = consts.tile([128, 1], F32)
        nc.vector.tensor_scalar(out=negthr_t, in0=thr_t, scalar1=-1.0,
                                scalar2=None, op0=ALU.mult)
        # bias for ACT-side tsm count: sigmoid(-1e8*c - 1e8*thr)
        bsig_t = consts.tile([128, 1], F32)
        nc.vector.tensor_scalar(out=bsig_t, in0=thr_t, scalar1=-1.0e8,
                                scalar2=None, op0=ALU.mult)

        tsmsum = slots.tile([128, NS], F32)
        osum_t = slots.tile([128, NS], F32)
        zero128_t = slots.tile([128, 128], BF16)
        nc.vector.memset(zero128_t, 0.0)
        nc.vector.memset(tsmsum, 0.0)
        edgesum = slots.tile([128, 6 * NS], F32)
        osb = slots.tile([12, 2048], F32)

        # one PSUM bank each, accumulated across all supertiles
        opc_t = ops.tile([12, 512], F32, name="opc")
        opt_t = ops.tile([12, 512], F32, name="opt")
        opm_t = ops.tile([12, 512], F32, name="opm")
        opq_t = ops.tile([12, 512], F32, name="opq")

        state = {}
        ctiles = []

        def front(s):
            c_t = io.tile([128, FW], BF16, tag="c", name=f"c{s}")
            nc.default_dma_engine.dma_start(
                out=c_t, in_=c_d[128 * s:128 * (s + 1), :])
            cc = c_t[:, C0:C0 + CW]
            c3 = c_t.rearrange("p (a b) -> p a b", b=W)
            o12 = ones_t[:, 12 * s:12 * (s + 1)]

            # sum(c) over center: ones-matmul accumulating across supertiles
            for k in range(9):
                nc.tensor.matmul(
                    out=opc_t, lhsT=o12,
                    rhs=c_t[:, C0 + 512 * k:C0 + 512 * (k + 1)],
                    start=(s == 0 and k == 0), stop=(s == NS - 1 and k == 8),
                    skip_group_check=True)

            # a = |c|: ACT Abs on center with fused sum(o); halo rows on DVE
            a_t = mida.tile([128, FW], BF16, tag="a", name=f"a{s}")
            a3 = a_t.rearrange("p (a b) -> p a b", b=W)
            nc.scalar.activation(
                out=a_t[:, C0:C0 + CW], in_=cc, func=ACTF.Abs,
                accum_out=osum_t[:, s:s + 1])
            nc.vector.scalar_tensor_tensor(
                out=a3[:, 0:1, :], in0=c3[:, 0:1, :], scalar=-1.0,
                in1=c3[:, 0:1, :], op0=ALU.mult, op1=ALU.max)
            nc.vector.scalar_tensor_tensor(
                out=a3[:, FH - 1:FH, :], in0=c3[:, FH - 1:FH, :], scalar=-1.0,
                in1=c3[:, FH - 1:FH, :], op0=ALU.mult, op1=ALU.max)

            # ts = (a > thr), whole tile in one 4x op (no accum!)
            ts_t = mida.tile([128, FW], BF16, tag="ts", name=f"ts{s}")
            ts3 = ts_t.rearrange("p (a b) -> p a b", b=W)
            nc.vector.tensor_scalar(
                out=ts_t, in0=a_t, scalar1=thr_t, scalar2=None,
                op0=ALU.is_gt)

            # d = W-derivative of ts (symmetric boundary), cols permuted:
            # cols 0..189 = d[w=1..190], col 190 = d[w=0], col 191 = d[w=191]
            d_t = midd.tile([128, FW], BF16, tag="d", name=f"d{s}")
            d3 = d_t.rearrange("p (a b) -> p a b", b=W)
            nc.vector.tensor_tensor(
                out=d3[:, :, 0:190], in0=ts3[:, :, 2:192],
                in1=ts3[:, :, 0:190], op=ALU.subtract)
            nc.vector.tensor_tensor(
                out=d3[:, :, 190:191], in0=ts3[:, :, 1:2],
                in1=ts3[:, :, 0:1], op=ALU.subtract)
            nc.vector.tensor_tensor(
                out=d3[:, :, 191:192], in0=ts3[:, :, 191:192],
                in1=ts3[:, :, 190:191], op=ALU.subtract)
            state[s] = (c_t, ts_t, d_t)
            ctiles.append(c_t)

        def back(s):
            c_t, ts_t, d_t = state.pop(s)
            cc = c_t[:, C0:C0 + CW]
            o12 = ones_t[:, 12 * s:12 * (s + 1)]

            # counts: materialize indicator at 4x, two exact bf16 pair-folds
            # at 2x, then a 3-matmul ones-reduce on PE (all correctly priced
            # by the scheduler's cost model, unlike fused DVE accum at 1x)
            def count_reduce(tag, op_psum, scalar1):
                ind = scr.tile([128, CW], BF16, tag="dum", name=f"{tag}i{s}")
                nc.vector.tensor_scalar(
                    out=ind, in0=cc, scalar1=scalar1, scalar2=None,
                    op0=ALU.is_lt)
                f1 = scr.tile([128, CW // 2], BF16, tag="f1", name=f"{tag}f{s}")
                nc.vector.tensor_tensor(
                    out=f1, in0=ind[:, 0:CW // 2], in1=ind[:, CW // 2:CW],
                    op=ALU.add)
                f2 = scr.tile([128, CW // 4], BF16, tag="f2", name=f"{tag}g{s}")
                nc.vector.tensor_tensor(
                    out=f2, in0=f1[:, 0:CW // 4], in1=f1[:, CW // 4:CW // 2],
                    op=ALU.add)
                for kk, (o0, w) in enumerate([(0, 512), (512, 512),
                                              (1024, 128)]):
                    nc.tensor.matmul(
                        out=op_psum[:, 0:w], lhsT=o12,
                        rhs=f2[:, o0:o0 + w],
                        start=(s == 0 and kk == 0),
                        stop=(s == NS - 1 and kk == 2),
                        skip_group_check=True)

            # taps feed the ACT edge passes (the busiest engine): keep them
            # at the head of the PE queue for this supertile
            # grad = S_z(S_h(d)) via 3 H-shifted banded matmuls into PSUM
            for j in range(6):
                g_t = gps.tile([128, GRP], F32, tag="g", name=f"g{s}_{j}")
                base = C0 + GRP * j
                for di, (lhs, doff) in enumerate(
                        [(bz1_t, -W), (bz1_t, W), (bz2_t, 0)]):
                    for k, (o0, w) in enumerate([(0, 512), (512, 256)]):
                        off = base + o0 + doff
                        nc.tensor.matmul(
                            out=g_t[:, o0:o0 + w],
                            lhsT=lhs, rhs=d_t[:, off:off + w],
                            start=(di == 0), stop=(di == 2))
                # edge = (grad > 0): integer grad, sigmoid saturates
                e_t = scr.tile([128, GRP], BF16, tag="edge", name=f"e{s}_{j}")
                nc.scalar.activation(
                    out=e_t, in_=g_t, func=ACTF.Sigmoid,
                    scale=100.0, bias=nbias_t,
                    accum_out=edgesum[:, 6 * s + j:6 * s + j + 1])
            # sum(ts) over center: ones-matmul accumulating across supertiles
            for k in range(9):
                nc.tensor.matmul(
                    out=opt_t, lhsT=o12,
                    rhs=ts_t[:, C0 + 512 * k:C0 + 512 * (k + 1)],
                    start=(s == 0 and k == 0), stop=(s == NS - 1 and k == 8),
                    skip_group_check=True)

            count_reduce("m", opm_t, 0.0)
            if s in ACT_TSM:
                dtsm = scr.tile([128, CW], BF16, tag="dum", name=f"dtsm{s}")
                nc.scalar.activation(
                    out=dtsm, in_=cc, func=ACTF.Sigmoid,
                    scale=-1.0e8, bias=bsig_t,
                    accum_out=tsmsum[:, s:s + 1])
                if s == NS - 1:
                    # close the opq accumulation group on the last supertile
                    nc.tensor.matmul(
                        out=opq_t[:, 0:128], lhsT=o12,
                        rhs=zero128_t, start=False, stop=True,
                        skip_group_check=True)
            else:
                count_reduce("q", opq_t, negthr_t)

        for s in range(NS):
            front(s)
            if s >= 1:
                back(s - 1)
        back(NS - 1)

        nc.scalar.copy(out=osb[:, 0:512], in_=opc_t)
        nc.scalar.copy(out=osb[:, 512:1024], in_=opt_t)
        nc.scalar.copy(out=osb[:, 1024:1536], in_=opm_t)
        nc.scalar.copy(out=osb[:, 1536:2048], in_=opq_t)

        nc.default_dma_engine.dma_start(out=part_d[:, 0:6], in_=osum_t)
        nc.default_dma_engine.dma_start(out=part_d[:, 6:12], in_=tsmsum)
        nc.default_dma_engine.dma_start(out=part_d[:, 12:48], in_=edgesum)
        nc.default_dma_engine.dma_start(out=osum_d, in_=osb)

    nc.compile()
    return nc


def _get_program():
    if "nc" not in _CACHE:
        _CACHE["nc"] = _build_program()
    return _CACHE["nc"]


def _make_in_maps(output, masks, loss_threshold):
    o = np.asarray(output, dtype=np.float32)
    m = np.asarray(masks, dtype=np.float32)
    c = np.copysign(o, np.float32(0.5) - m).astype(ml_dtypes.bfloat16)
    c5 = c.reshape(NV, Z, H, W)
    thr = np.full((1, 1), np.float32(np.asarray(loss_threshold)), np.float32)
    bz1, bz2, ones12 = _consts()
    in_maps = []
    for cid in range(NCORES):
        h0 = HC * cid
        idx = np.clip(np.arange(h0 - 1, h0 + HC + 1), 0, H - 1)
        c_sh = np.ascontiguousarray(c5[:, :, idx, :]).reshape(NV * Z, FW)
        in_maps.append({
            "c": c_sh, "thr": thr,
            "bz1": bz1, "bz2": bz2, "ones12": ones12,
        })
    return in_maps


def _combine(results):
    """Host-side tiny reduction: per-core partials -> loss scalar."""
    sum_ts = np.zeros(NV)
    sum_tsm = np.zeros(NV)
    sum_m = np.zeros(NV)
    sum_o = np.zeros(NV)
    sum_edge = np.zeros(NV)
    sum_c = np.zeros(NV)
    for r in results:
        p = np.asarray(r["partials"], dtype=np.float64)
        osum = np.asarray(r["osum"], dtype=np.float64)
        # [partition, s]: volume = 2s + partition//64, z = partition%64
        sum_o += p[:, 0:6].reshape(2, 64, NS).sum(1).T.reshape(-1)
        sum_tsm += p[:, 6:12].reshape(2, 64, NS).sum(1).T.reshape(-1)
        sum_edge += (p[:, 12:48].reshape(2, 64, NS, 6).sum(axis=(1, 3))
                     .T.reshape(-1))
        # osum rows are volumes directly (block ones lhsT)
        sum_c += osum[:, 0:512].sum(-1)
        sum_ts += osum[:, 512:1024].sum(-1)
        sum_m += osum[:, 1024:1536].sum(-1)
        sum_tsm += osum[:, 1536:2048].sum(-1)

    sum_om = 0.5 * (sum_o - sum_c)
    sum_eq = VOX - sum_ts - sum_m + 2.0 * sum_tsm

    freq = (sum_m / VOX).reshape(B, C)
    med = np.median(freq, axis=1, keepdims=True)
    w0 = 2.0 * med / (freq.min(axis=1, keepdims=True) + 1e-5)
    cw = (med / (freq + 1e-5)) * sum_eq.reshape(B, C) \
        + w0 * sum_edge.reshape(B, C)
    ps1 = sum_om.reshape(B, C)
    ps2 = (sum_o + sum_m).reshape(B, C)
    nom = (cw * ps1).sum(1)
    denom = (cw * ps2 + 1e-7).sum(1)
    loss = (1.0 - 2.0 * nom / denom).sum() / B
    return np.array([loss], dtype=np.float32)


def run(output, masks, loss_threshold, trace=False, **trace_kwargs):
    nc = _get_program()
    in_maps = _make_in_maps(output, masks, loss_threshold)
    res = run_bass_kernel_spmd(nc, in_maps, list(range(NCORES)),
                               trace=trace, **trace_kwargs)
    return _combine(res.results), res


def kernel(output, masks, loss_threshold):
    loss, _ = run(output, masks, loss_threshold)
    return loss


# revision 27
# speedup vs baseline: 1.1898x; 1.0385x over previous
"""Trainium2 Bass kernel for LogWeightedDICELossMultiClass3D.

Input: output (4,3,64,192,192) f32, masks (same), loss_threshold scalar.

Strategy: host packs both inputs into ONE bf16 tensor
    c = copysign(output, 0.5 - masks)        (sign bit = mask, |c| = output)
so DMA traffic is 2 bytes/voxel instead of 8. H=192 is sharded into 8
slabs of 24 rows (one per core, 1-row halo clamped on host).

Per core, 6 supertiles of 128 partitions (= 2 volumes x 64 z), free dim =
26 H-rows x 192 W. Reductions are expensive on this HW (any DVE op with
accum_out drops to 1x; ACT is 1 elem/cycle; PE ones-matmul is 1 col/cycle)
so each of the six per-(vol,z)/per-volume sums rides the cheapest slot:
  sum_o    : fused accum of the ACT Abs pass that materializes a=|c|
  sum_c    : PE ones-matmul on c  (sum_om = (sum_o - sum_c)/2 on host)
  sum_ts   : PE ones-matmul on ts (both accumulate over all supertiles
             into one [12,512] PSUM tile via a block ones lhsT)
  sum_m    : DVE is_lt + add-accum (1x)
  sum_tsm  : DVE is_lt + add-accum for 3 supertiles, ACT saturated
             sigmoid for 3 (engine balance)
  sum_edge : ts = (a > thr) on DVE (4x); d = W-deriv(ts) on DVE (2x);
             H-smooth (3 shifted taps) x Z-smooth (block-banded lhsT) on
             PE into PSUM; edge = (grad > 0) via saturated Sigmoid on
             ACT + accum (grad is integer-valued so this is exact)
Host combines the tiny partials into the scalar loss
(sum_eq = vox - sum_ts - sum_m + 2*sum_tsm).
"""

import numpy as np
import ml_dtypes

import concourse.bacc as bacc
import concourse.bass as bass
import concourse.tile as tile
from concourse import mybir
from concourse.bass_utils import run_bass_kernel_spmd

F32 = mybir.dt.float32
BF16 = mybir.dt.bfloat16
ALU = mybir.AluOpType
ACTF = mybir.ActivationFunctionType

B, C, Z, H, W = 4, 3, 64, 192, 192
NV = B * C            # 12 volumes
NCORES = 8
HC = H // NCORES      # 24 H-rows per core
NS = NV // 2          # 6 supertiles (2 volumes each)
FH = HC + 2           # 26 rows incl halo
FW = FH * W           # 4992 free elements per partition of c / a / ts / d
CW = HC * W           # 4608 center free elements
C0 = W                # flat offset of center region (row 1)
GRP = 768             # tap-group width (2 PSUM banks)
VOX = Z * H * W
ACT_TSM = (4, 5)      # supertiles whose tsm-count runs on ACT, not DVE

_CACHE = {}


def _band64():
    """[1,2,1] Z-smoothing matrix with scipy 'reflect' ends."""
    M = np.zeros((Z, Z), dtype=np.float64)
    for i in range(Z):
        M[i, i] = 2.0
        M[i, max(i - 1, 0)] += 1.0
        M[i, min(i + 1, Z - 1)] += 1.0
    return M


def _consts():
    Bz = _band64()
    blk = np.zeros((128, 128), dtype=np.float64)
    blk[:64, :64] = Bz
    blk[64:, 64:] = Bz
    bz1 = blk.astype(ml_dtypes.bfloat16)          # weights 1,2,3 - exact
    bz2 = (2.0 * blk).astype(ml_dtypes.bfloat16)  # weights 2,4,6 - exact
    # per-supertile [128,12] ones blocks: slice s has col 2s = ones on
    # partitions 0..63 (volume 2s) and col 2s+1 = ones on 64..127.
    ones12 = np.zeros((128, 12 * NS), dtype=ml_dtypes.bfloat16)
    for s in range(NS):
        ones12[:64, 12 * s + 2 * s] = 1.0
        ones12[64:, 12 * s + 2 * s + 1] = 1.0
    return bz1, bz2, ones12


def _build_program():
    nc = bacc.Bacc("TRN2", target_bir_lowering=False, debug=False,
                   num_devices=NCORES)
    c_d = nc.dram_tensor("c", [NV * Z, FW], BF16, kind="ExternalInput").ap()
    thr_d = nc.dram_tensor("thr", [1, 1], F32, kind="ExternalInput").ap()
    bz1_d = nc.dram_tensor("bz1", [128, 128], BF16, kind="ExternalInput").ap()
    bz2_d = nc.dram_tensor("bz2", [128, 128], BF16, kind="ExternalInput").ap()
    ones_d = nc.dram_tensor("ones12", [128, 12 * NS], BF16,
                            kind="ExternalInput").ap()
    part_d = nc.dram_tensor("partials", [128, 48], F32,
                            kind="ExternalOutput").ap()
    osum_d = nc.dram_tensor("osum", [12, 2048], F32,
                            kind="ExternalOutput").ap()

    from contextlib import ExitStack
    with tile.TileContext(nc) as tc, ExitStack() as ctx:
        consts = ctx.enter_context(tc.tile_pool(name="consts", bufs=1))
        io = ctx.enter_context(tc.tile_pool(name="io", bufs=4))
        mida = ctx.enter_context(tc.tile_pool(name="mida", bufs=3))
        midd = ctx.enter_context(tc.tile_pool(name="midd", bufs=3))
        scr = ctx.enter_context(tc.tile_pool(name="scr", bufs=3))
        slots = ctx.enter_context(tc.tile_pool(name="slots", bufs=1))
        gps = ctx.enter_context(tc.tile_pool(name="gps", bufs=2, space="PSUM"))
        ops = ctx.enter_context(tc.tile_pool(name="ops", bufs=1, space="PSUM"))

        thr_t = consts.tile([128, 1], F32)
        nc.gpsimd.dma_start(out=thr_t, in_=thr_d.to_broadcast([128, 1]))
        bz1_t = consts.tile([128, 128], BF16)
        nc.default_dma_engine.dma_start(out=bz1_t, in_=bz1_d)
        bz2_t = consts.tile([128, 128], BF16)
        nc.default_dma_engine.dma_start(out=bz2_t, in_=bz2_d)
        ones_t = consts.tile([128, 12 * NS], BF16)
        nc.default_dma_engine.dma_start(out=ones_t, in_=ones_d)
        nbias_t = consts.tile([128, 1], F32)
        nc.vector.memset(nbias_t, -50.0)
        negthr_t = consts.tile([128, 1], F32)
        nc.vector.tensor_scalar(out=negthr_t, in0=thr_t, scalar1=-1.0,
                                scalar2=None, op0=ALU.mult)
        # bias for ACT-side tsm count: sigmoid(-1e8*c - 1e8*thr)
        bsig_t = consts.tile([128, 1], F32)
        nc.vector.tensor_scalar(out=bsig_t, in0=thr_t, scalar1=-1.0e8,
                                scalar2=None, op0=ALU.mult)

        tsmsum = slots.tile([128, NS], F32)
        osum_t = slots.tile([128, NS], F32)
        zero128_t = slots.tile([128, 128], BF16)
        nc.vector.memset(zero128_t, 0.0)
        nc.vector.memset(tsmsum, 0.0)
        edgesum = slots.tile([128, 6 * NS], F32)
        osb = slots.tile([12, 2048], F32)

        # one PSUM bank each, accumulated across all supertiles
        opc_t = ops.tile([12, 512], F32, name="opc")
        opt_t = ops.tile([12, 512], F32, name="opt")
        opm_t = ops.tile([12, 512], F32, name="opm")
        opq_t = ops.tile([12, 512], F32, name="opq")

        state = {}
        ctiles = []

        def front(s):
            c_t = io.tile([128, FW], BF16, tag="c", name=f"c{s}")
            nc.default_dma_engine.dma_start(
                out=c_t, in_=c_d[128 * s:128 * (s + 1), :])
            cc = c_t[:, C0:C0 + CW]
            c3 = c_t.rearrange("p (a b) -> p a b", b=W)
            o12 = ones_t[:, 12 * s:12 * (s + 1)]

            # sum(c) over center: ones-matmul accumulating across supertiles
            for k in range(9):
                nc.tensor.matmul(
                    out=opc_t, lhsT=o12,
                    rhs=c_t[:, C0 + 512 * k:C0 + 512 * (k + 1)],
                    start=(s == 0 and k == 0), stop=(s == NS - 1 and k == 8),
                    skip_group_check=True)

            # a = |c|: ACT Abs on center with fused sum(o); halo rows on DVE
            a_t = mida.tile([128, FW], BF16, tag="a", name=f"a{s}")
            a3 = a_t.rearrange("p (a b) -> p a b", b=W)
            nc.scalar.activation(
                out=a_t[:, C0:C0 + CW], in_=cc, func=ACTF.Abs,
                accum_out=osum_t[:, s:s + 1])
            nc.vector.scalar_tensor_tensor(
                out=a3[:, 0:1, :], in0=c3[:, 0:1, :], scalar=-1.0,
                in1=c3[:, 0:1, :], op0=ALU.mult, op1=ALU.max)
            nc.vector.scalar_tensor_tensor(
                out=a3[:, FH - 1:FH, :], in0=c3[:, FH - 1:FH, :], scalar=-1.0,
                in1=c3[:, FH - 1:FH, :], op0=ALU.mult, op1=ALU.max)

            # ts = (a > thr), whole tile in one 4x op (no accum!)
            ts_t = mida.tile([128, FW], BF16, tag="ts", name=f"ts{s}")
            ts3 = ts_t.rearrange("p (a b) -> p a b", b=W)
            nc.vector.tensor_scalar(
                out=ts_t, in0=a_t, scalar1=thr_t, scalar2=None,
                op0=ALU.is_gt)

            # d = W-derivative of ts (symmetric boundary), cols permuted:
            # cols 0..189 = d[w=1..190], col 190 = d[w=0], col 191 = d[w=191]
            d_t = midd.tile([128, FW], BF16, tag="d", name=f"d{s}")
            d3 = d_t.rearrange("p (a b) -> p a b", b=W)
            nc.vector.tensor_tensor(
                out=d3[:, :, 0:190], in0=ts3[:, :, 2:192],
                in1=ts3[:, :, 0:190], op=ALU.subtract)
            nc.vector.tensor_tensor(
                out=d3[:, :, 190:191], in0=ts3[:, :, 1:2],
                in1=ts3[:, :, 0:1], op=ALU.subtract)
            nc.vector.tensor_tensor(
                out=d3[:, :, 191:192], in0=ts3[:, :, 191:192],
                in1=ts3[:, :, 190:191], op=ALU.subtract)
            state[s] = (c_t, ts_t, d_t)
            ctiles.append(c_t)

        def back(s):
            c_t, ts_t, d_t = state.pop(s)
            cc = c_t[:, C0:C0 + CW]
            o12 = ones_t[:, 12 * s:12 * (s + 1)]

            # counts: materialize indicator at 4x, two exact bf16 pair-folds
            # at 2x, then a 3-matmul ones-reduce on PE (all correctly priced
            # by the scheduler's cost model, unlike fused DVE accum at 1x)
            def count_reduce(tag, op_psum, scalar1):
                ind = scr.tile([128, CW], BF16, tag="dum", name=f"{tag}i{s}")
                nc.vector.tensor_scalar(
                    out=ind, in0=cc, scalar1=scalar1, scalar2=None,
                    op0=ALU.is_lt)
                f1 = scr.tile([128, CW // 2], BF16, tag="f1", name=f"{tag}f{s}")
                nc.vector.tensor_tensor(
                    out=f1, in0=ind[:, 0:CW // 2], in1=ind[:, CW // 2:CW],
                    op=ALU.add)
                f2 = scr.tile([128, CW // 4], BF16, tag="f2", name=f"{tag}g{s}")
                nc.vector.tensor_tensor(
                    out=f2, in0=f1[:, 0:CW // 4], in1=f1[:, CW // 4:CW // 2],
                    op=ALU.add)
                for kk, (o0, w) in enumerate([(0, 512), (512, 512),
                                              (1024, 128)]):
                    nc.tensor.matmul(
                        out=op_psum[:, 0:w], lhsT=o12,
                        rhs=f2[:, o0:o0 + w],
                        start=(s == 0 and kk == 0),
                        stop=(s == NS - 1 and kk == 2),
                        skip_group_check=True)

            # taps feed the ACT edge passes (the busiest engine): keep them
            # at the head of the PE queue for this supertile
            # grad = S_z(S_h(d)) via 3 H-shifted banded matmuls into PSUM
            for j in range(6):
                g_t = gps.tile([128, GRP], F32, tag="g", name=f"g{s}_{j}")
                base = C0 + GRP * j
                for di, (lhs, doff) in enumerate(
                        [(bz1_t, -W), (bz1_t, W), (bz2_t, 0)]):
                    for k, (o0, w) in enumerate([(0, 512), (512, 256)]):
                        off = base + o0 + doff
                        nc.tensor.matmul(
                            out=g_t[:, o0:o0 + w],
                            lhsT=lhs, rhs=d_t[:, off:off + w],
                            start=(di == 0), stop=(di == 2))
                # edge = (grad > 0): exact for integer grad. Last two
                # supertiles count on DVE (is_gt + add-reduce) to shorten
                # the ACT tail; earlier ones use the ACT sigmoid trick.
                e_t = scr.tile([128, GRP], BF16, tag="edge", name=f"e{s}_{j}")
                if s >= 4:
                    nc.vector.tensor_scalar(
                        out=e_t, in0=g_t, scalar1=0.0, scalar2=None,
                        op0=ALU.is_gt, op1=ALU.add,
                        accum_out=edgesum[:, 6 * s + j:6 * s + j + 1])
                else:
                    nc.scalar.activation(
                        out=e_t, in_=g_t, func=ACTF.Sigmoid,
                        scale=100.0, bias=nbias_t,
                        accum_out=edgesum[:, 6 * s + j:6 * s + j + 1])
            # sum(ts) over center: ones-matmul accumulating across supertiles
            for k in range(9):
                nc.tensor.matmul(
                    out=opt_t, lhsT=o12,
                    rhs=ts_t[:, C0 + 512 * k:C0 + 512 * (k + 1)],
                    start=(s == 0 and k == 0), stop=(s == NS - 1 and k == 8),
                    skip_group_check=True)

            count_reduce("m", opm_t, 0.0)
            if s in ACT_TSM:
                dtsm = scr.tile([128, CW], BF16, tag="dum", name=f"dtsm{s}")
                nc.scalar.activation(
                    out=dtsm, in_=cc, func=ACTF.Sigmoid,
                    scale=-1.0e8, bias=bsig_t,
                    accum_out=tsmsum[:, s:s + 1])
                if s == NS - 1:
                    # close the opq accumulation group on the last supertile
                    nc.tensor.matmul(
                        out=opq_t[:, 0:128], lhsT=o12,
                        rhs=zero128_t, start=False, stop=True,
                        skip_group_check=True)
            else:
                count_reduce("q", opq_t, negthr_t)

        for s in range(NS):
            front(s)
            if s >= 1:
                back(s - 1)
        back(NS - 1)

        nc.scalar.copy(out=osb[:, 0:512], in_=opc_t)
        nc.scalar.copy(out=osb[:, 512:1024], in_=opt_t)
        nc.scalar.copy(out=osb[:, 1024:1536], in_=opm_t)
        nc.scalar.copy(out=osb[:, 1536:2048], in_=opq_t)

        nc.default_dma_engine.dma_start(out=part_d[:, 0:6], in_=osum_t)
        nc.default_dma_engine.dma_start(out=part_d[:, 6:12], in_=tsmsum)
        nc.default_dma_engine.dma_start(out=part_d[:, 12:48], in_=edgesum)
        nc.default_dma_engine.dma_start(out=osum_d, in_=osb)

    nc.compile()
    return nc


def _get_program():
    if "nc" not in _CACHE:
        _CACHE["nc"] = _build_program()
    return _CACHE["nc"]


def _make_in_maps(output, masks, loss_threshold):
    o = np.asarray(output, dtype=np.float32)
    m = np.asarray(masks, dtype=np.float32)
    c = np.copysign(o, np.float32(0.5) - m).astype(ml_dtypes.bfloat16)
    c5 = c.reshape(NV, Z, H, W)
    thr = np.full((1, 1), np.float32(np.asarray(loss_threshold)), np.float32)
    bz1, bz2, ones12 = _consts()
    in_maps = []
    for cid in range(NCORES):
        h0 = HC * cid
        idx = np.clip(np.arange(h0 - 1, h0 + HC + 1), 0, H - 1)
        c_sh = np.ascontiguousarray(c5[:, :, idx, :]).reshape(NV * Z, FW)
        in_maps.append({
            "c": c_sh, "thr": thr,
            "bz1": bz1, "bz2": bz2, "ones12": ones12,
        })
    return in_maps


def _combine(results):
    """Host-side tiny reduction: per-core partials -> loss scalar."""
    sum_ts = np.zeros(NV)
    sum_tsm = np.zeros(NV)
    sum_m = np.zeros(NV)
    sum_o = np.zeros(NV)
    sum_edge = np.zeros(NV)
    sum_c = np.zeros(NV)
    for r in results:
        p = np.asarray(r["partials"], dtype=np.float64)
        osum = np.asarray(r["osum"], dtype=np.float64)
        # [partition, s]: volume = 2s + partition//64, z = partition%64
        sum_o += p[:, 0:6].reshape(2, 64, NS).sum(1).T.reshape(-1)
        sum_tsm += p[:, 6:12].reshape(2, 64, NS).sum(1).T.reshape(-1)
        sum_edge += (p[:, 12:48].reshape(2, 64, NS, 6).sum(axis=(1, 3))
                     .T.reshape(-1))
        # osum rows are volumes directly (block ones lhsT)
        sum_c += osum[:, 0:512].sum(-1)
        sum_ts += osum[:, 512:1024].sum(-1)
        sum_m += osum[:, 1024:1536].sum(-1)
        sum_tsm += osum[:, 1536:2048].sum(-1)

    sum_om = 0.5 * (sum_o - sum_c)
    sum_eq = VOX - sum_ts - sum_m + 2.0 * sum_tsm

    freq = (sum_m / VOX).reshape(B, C)
    med = np.median(freq, axis=1, keepdims=True)
    w0 = 2.0 * med / (freq.min(axis=1, keepdims=True) + 1e-5)
    cw = (med / (freq + 1e-5)) * sum_eq.reshape(B, C) \
        + w0 * sum_edge.reshape(B, C)
    ps1 = sum_om.reshape(B, C)
    ps2 = (sum_o + sum_m).reshape(B, C)
    nom = (cw * ps1).sum(1)
    denom = (cw * ps2 + 1e-7).sum(1)
    loss = (1.0 - 2.0 * nom / denom).sum() / B
    return np.array([loss], dtype=np.float32)


def run(output, masks, loss_threshold, trace=False, **trace_kwargs):
    nc = _get_program()
    in_maps = _make_in_maps(output, masks, loss_threshold)
    res = run_bass_kernel_spmd(nc, in_maps, list(range(NCORES)),
                               trace=trace, **trace_kwargs)
    return _combine(res.results), res


def kernel(output, masks, loss_threshold):
    loss, _ = run(output, masks, loss_threshold)
    return loss
